# revision 20
# baseline (speedup 1.0000x reference)
"""LightGCN-style GNN message passing (n_layers x SpMM + running mean) on 8 TRN2
NeuronCores.

Row-parallel SpMM (1D graph partition):
  - Core c owns dest-node slab [c*SLAB, (c+1)*SLAB).
  - Src nodes are split into C buckets == the C slabs, so every dma_gather index
    is slab-local (< 18750, fits the int16 index requirement).
  - Work unit = (dest group g of GR nodes, src bucket k).  Host builds a padded
    CSR per unit with nodes degree-sorted into blocks of 128 and a GLOBAL
    (core-max) block-degree profile so the SPMD graph is identical on all cores.
  - Per unit: per-edge dma_gather (f32 rows, 256 B) from the bucket table ->
    DVE multiply by vals -> DVE strided segment-reduce per equal-degree block
    run -> dma_scatter_add merges the bucket partial into the HBM y slab.
  - Per dest group g one small AllGather broadcasts all cores' group-g rows;
    a local re-stripe DMA lays them out slab-contiguously as next layer's
    gather tables.  The C small AllGathers per layer pipeline behind compute.
  - acc (running sum over layer outputs) lives in SBUF f32; final scale by
    1/(n_layers+1); host unscrambles the partition-major output layout.
"""

import sys
import numpy as np

if "/opt/trn_rl_repo" not in sys.path:
    sys.path.insert(0, "/opt/trn_rl_repo")

USER_NUM = 100000
ITEM_NUM = 50000


def make_cfg(n_nodes=150000, emb=64, n_cores=8, n_groups=8, wmax=104):
    slab = n_nodes // n_cores
    assert slab * n_cores == n_nodes
    gr = -(-slab // n_groups)
    ranks = -(-gr // 128) * 128
    return dict(N=n_nodes, EMB=emb, C=n_cores, NG=n_groups, GR=gr, RANKS=ranks,
                BL=ranks // 128, SLAB=slab, SLABP=gr * n_groups, WMAX=wmax)


# ---------------------------------------------------------------------------
# host-side plan
# ---------------------------------------------------------------------------

def build_plan(rows, cols, vals, cfg):
    C, NG, GR, RANKS, BL, SLAB, WMAX = (
        cfg["C"], cfg["NG"], cfg["GR"], cfg["RANKS"], cfg["BL"], cfg["SLAB"],
        cfg["WMAX"])
    NU = NG * C

    rows = np.asarray(rows, dtype=np.int64)
    cols = np.asarray(cols, dtype=np.int64)
    vals = np.asarray(vals, dtype=np.float32)
    c = rows // SLAB
    loc = rows - c * SLAB
    g = np.minimum(loc // GR, NG - 1)
    grow = loc - g * GR
    k = cols // SLAB
    colloc = (cols - k * SLAB).astype(np.int16)

    cu = (c * NU + g * C + k).astype(np.int64)
    nkey = cu * GR + grow

    deg = np.bincount(nkey, minlength=C * NU * GR).reshape(C * NU, GR)
    order = np.argsort(-deg, axis=1, kind="stable")          # rank -> node id
    rank_of = np.empty_like(order)
    np.put_along_axis(rank_of, order,
                      np.broadcast_to(np.arange(GR), (C * NU, GR)), axis=1)

    ds = np.zeros((C * NU, RANKS), dtype=np.int64)
    ds[:, :GR] = np.take_along_axis(deg, order, axis=1)
    bmax = ds.reshape(C * NU, BL, 128).max(axis=2)
    Dprof = bmax.reshape(C, NU, BL).max(axis=0)              # [NU, BL]
    assert Dprof.max() <= WMAX, f"block degree {Dprof.max()} > WMAX {WMAX}"

    # tile packing + global slot offsets per block
    glob_blk_slot = np.zeros((NU, BL), dtype=np.int64)
    units = []
    tot_slots = 0
    for u in range(NU):
        gg, kk = divmod(u, C)
        nbl = int((Dprof[u] > 0).sum())      # zero-D blocks only at the end
        tiles = []
        b = 0
        while b < nbl:
            slots = 0
            runs = []
            tile_off = tot_slots
            b0 = b
            while b < nbl and slots + int(Dprof[u, b]) <= WMAX:
                D = int(Dprof[u, b])
                glob_blk_slot[u, b] = tile_off + slots
                if runs and runs[-1][0] == D:
                    runs[-1][1] += 1
                else:
                    runs.append([D, 1])
                slots += D
                b += 1
            tiles.append(dict(slots=slots, runs=[tuple(r) for r in runs],
                              b0=b0, slot_off=tile_off))
            tot_slots += slots
        units.append(dict(g=gg, k=kk, uid=u, tiles=tiles))

    # within-node edge counter j
    es = np.argsort(nkey, kind="stable")
    nk_s = nkey[es]
    first = np.r_[0, np.flatnonzero(np.diff(nk_s)) + 1]
    starts = np.zeros(len(nk_s), dtype=np.int64)
    starts[first] = first
    starts = np.maximum.accumulate(starts)
    j = np.empty_like(starts)
    j[es] = np.arange(len(nk_s)) - starts

    r_e = rank_of[cu, grow]
    u_e = cu % NU
    slot_e = glob_blk_slot[u_e, r_e // 128] + j
    pos = slot_e * 128 + (r_e % 128)

    gidx_lin = np.zeros((C, tot_slots * 128), dtype=np.int16)
    gval_lin = np.zeros((C, tot_slots * 128), dtype=np.float32)
    gidx_lin[c, pos] = colloc
    gval_lin[c, pos] = vals

    gval_w = np.ascontiguousarray(
        gval_lin.reshape(C, tot_slots, 128).transpose(0, 2, 1))

    # wrap gather idx per tile -> [C, 128, tot_slots*8]
    gidx_w = np.zeros((C, 128, tot_slots * 8), dtype=np.int16)
    off16 = 0
    for u in range(NU):
        for t in units[u]["tiles"]:
            ni = t["slots"] * 128
            a = t["slot_off"] * 128
            seg = gidx_lin[:, a:a + ni].reshape(C, ni // 16, 16)
            gidx_w[:, :16, off16:off16 + ni // 16] = seg.transpose(0, 2, 1)
            t["ni_off16"] = off16
            off16 += ni // 16
    gidx_w[:, 16:, :] = np.tile(gidx_w[:, :16, :], (1, 7, 1))

    # scatter idx per unit
    sidx_lin = np.full((C, NU, RANKS), -1, dtype=np.int16)
    sidx_lin[:, :, :GR] = order.reshape(C, NU, GR).astype(np.int16)
    seg = sidx_lin.reshape(C, NU, RANKS // 16, 16).transpose(0, 1, 3, 2)
    sidx_w = np.broadcast_to(seg[:, :, None, :, :],
                             (C, NU, 8, 16, RANKS // 16))
    sidx_w = np.ascontiguousarray(
        sidx_w.reshape(C, NU, 128, RANKS // 16).transpose(0, 2, 1, 3)
        .reshape(C, 128, NU * (RANKS // 16)))

    return dict(units=units, tot_slots=tot_slots, tot_ni16=off16,
                gidx=gidx_w, gval=gval_w, sidx=sidx_w)


# ---------------------------------------------------------------------------
# schedule (engine-op list with cumulative semaphore targets)
# ---------------------------------------------------------------------------

def build_sched(plan, cfg, n_layers):
    """Emit a per-engine op list.  Semaphore discipline: every wait value is
    the total-so-far of its semaphore, and any two DMAs sharing a semaphore
    are serialized by a consumer dependency, so thresholds are race-free."""
    C, NG = cfg["C"], cfg["NG"]
    units = plan["units"]
    tiles_flat = []
    for u in units:
        for ti, t in enumerate(u["tiles"]):
            tiles_flat.append((u, ti, t))
    TPL = len(tiles_flat)
    NT = TPL * n_layers

    SEMS = (["g0", "g1", "sx0", "sx1", "ss", "c0", "c1", "is", "a0", "a1",
             "r0", "r1", "v", "os"] + [f"z{g}" for g in range(NG)])
    cnt = {s: 0 for s in SEMS}
    sched = []

    def emit(eng, kind, waits=(), inc=None, **kw):
        w = {}
        for sname, val in waits:
            w[sname] = max(w.get(sname, 0), val)
        sched.append(dict(eng=eng, kind=kind,
                          waits=[(s_, v_) for s_, v_ in w.items() if v_ > 0],
                          inc_sem=inc[0] if inc else None, **kw))
        if inc:
            cnt[inc[0]] += inc[1]

    v_after_tile = {}
    g_after_tile = {}
    sx_after_stream = {}
    s_after_unit = {}
    r_after = {}
    c_after_ag = {}
    v_after_accadd = {}
    pending = []

    emit("g", "lib")
    emit("x", "dma", dst=("gval_s", None), src=("gval_in", None), inc=("is", 16))
    emit("x", "dma", dst=("sidx_s", None), src=("sidx_in", None), inc=("is", 16))
    emit("x", "dma", dst=("acc", None), src=("xslab", None), inc=("is", 16))
    emit("v", "memset", inc=("v", 1))
    zero_v = cnt["v"]
    if n_layers > 0:
        for gg in range(NG):
            emit("x", "dma", dst=("ybounce_chunk", gg), src=("zeros", None),
                 waits=[("v", zero_v)], inc=(f"z{gg}", 16))

    def stream(tg):
        if tg >= NT:
            return
        u, ti, t = tiles_flat[tg % TPL]
        p = tg % 2
        waits = []
        if tg >= 2:
            waits.append((f"g{p}", g_after_tile[tg - 2]))
        emit("x", "dma", dst=("gix", (p, t["slots"] * 8)),
             src=("gidx_slice", (t["ni_off16"], t["slots"] * 8)),
             waits=waits, inc=(f"sx{p}", 16))
        sx_after_stream[tg] = cnt[f"sx{p}"]

    def flush(cur_key):
        while pending and pending[0][0] <= cur_key - 2:
            _, fn = pending.pop(0)
            fn()

    stream(0)
    stream(1)

    for L in range(n_layers):
        flush(L * NG + 1)
        for gt, (u, ti, t) in enumerate(tiles_flat):
            tg = L * TPL + gt
            p = tg % 2
            uid, gg, kk = u["uid"], u["g"], u["k"]
            flush(L * NG + gg)

            waits = [(f"sx{p}", sx_after_stream[tg])]
            if tg >= 2:
                waits.append(("v", v_after_tile[tg - 2]))
            if L >= 1:
                waits.append(("r0", r_after.get(("r0", L - 1), 0)))
                waits.append(("r1", r_after.get(("r1", L - 1), 0)))
            emit("g", "gather", tile=p, slots=t["slots"], layer=L,
                 bucket=kk, ni=t["slots"] * 128, waits=waits,
                 inc=(f"g{p}", 16))
            g_after_tile[tg] = cnt[f"g{p}"]

            stream(tg + 2)

            waits = [(f"g{p}", g_after_tile[tg])]
            if tg == 0:
                waits.append(("is", 48))
            emit("v", "mult", tile=p, slots=t["slots"], voff=t["slot_off"],
                 waits=waits, inc=("v", 1))
            first_red = (ti == 0)
            c0 = 0
            b = t["b0"]
            for (D, nb) in t["runs"]:
                w = [("ss", cnt["ss"])] if first_red else []
                first_red = False
                emit("v", "reduce", tile=p, c0=c0, D=D, nb=nb, b0=b,
                     waits=w, inc=("v", 1))
                c0 += D * nb
                b += nb
            v_after_tile[tg] = cnt["v"]

            if ti == len(u["tiles"]) - 1:
                waits = [("v", v_after_tile[tg]),
                         (f"z{gg}", 16 * (L + 1)),
                         ("ss", cnt["ss"])]
                if cnt["ss"] == 0:
                    waits.append(("is", 48))
                emit("g", "scatter", uid=uid, g=gg, waits=waits,
                     inc=("ss", 16))
                s_after_unit[(L, uid)] = cnt["ss"]

                if kk == C - 1:
                    if L < n_layers - 1:
                        agk = L * NG + gg
                        waits = [("ss", cnt["ss"])]
                        if agk >= 2:
                            waits.append((f"r{agk % 2}",
                                          16 * (agk // 2)))
                        emit("g", "ag", g=gg, buf=agk % 2, waits=waits,
                             inc=(f"c{agk % 2}", 1))
                        c_after_ag[(L, gg)] = cnt[f"c{agk % 2}"]

                    def mk(Lc, gc):
                        def fn():
                            pa = (Lc * NG + gc) % 2
                            w = [("ss", s_after_unit[(Lc, gc * C + C - 1)])]
                            prev = (Lc * NG + gc) - 2
                            if prev >= 0:
                                Lp, gp = divmod(prev, NG)
                                if (Lp, gp) in v_after_accadd:
                                    w.append(("v", v_after_accadd[(Lp, gp)]))
                            emit("x", "dma", dst=("atmp", pa),
                                 src=("ybounce_chunk", gc), waits=w,
                                 inc=(f"a{pa}", 16))
                            emit("v", "accadd", g=gc, buf=pa,
                                 waits=[(f"a{pa}", cnt[f"a{pa}"])],
                                 inc=("v", 1))
                            v_after_accadd[(Lc, gc)] = cnt["v"]
                            if Lc < n_layers - 1:
                                agk = Lc * NG + gc
                                emit("x", "dma", dst=("xn_stripe", (Lc, gc)),
                                     src=("agt", agk % 2),
                                     waits=[(f"c{agk % 2}",
                                             c_after_ag[(Lc, gc)])],
                                     inc=(f"r{agk % 2}", 16))
                                r_after[(f"r{agk % 2}", Lc)] = \
                                    cnt[f"r{agk % 2}"]
                                emit("x", "dma", dst=("ybounce_chunk", gc),
                                     src=("zeros", None),
                                     waits=[(f"a{pa}", cnt[f"a{pa}"])],
                                     inc=(f"z{gc}", 16))
                        return fn

                    pending.append((L * NG + gg, mk(L, gg)))

    flush(10 ** 9)
    emit("v", "scale", factor=1.0 / float(n_layers + 1), inc=("v", 1))
    emit("x", "dma", dst=("out", None), src=("acc", None),
         waits=[("v", cnt["v"])], inc=("os", 16))
    emit("x", "wait", waits=[("os", cnt["os"])])
    emit("g", "wait", waits=[("ss", cnt["ss"]), ("g0", cnt["g0"]),
                             ("g1", cnt["g1"])])
    return dict(sched=sched, sems=SEMS)


# ---------------------------------------------------------------------------
# bass graph
# ---------------------------------------------------------------------------

def build_nc(plan, cfg, n_layers, detect_races=True):
    import concourse.bass as bass
    import concourse.bacc as bacc
    import concourse.mybir as mybir
    from concourse.library_config import mlp

    C, NG, GR, RANKS, BL, SLAB, SLABP, EMB, WMAX, N = (
        cfg["C"], cfg["NG"], cfg["GR"], cfg["RANKS"], cfg["BL"], cfg["SLAB"],
        cfg["SLABP"], cfg["EMB"], cfg["WMAX"], cfg["N"])
    NU = NG * C
    FP, I16 = mybir.dt.float32, mybir.dt.int16
    TOTS, TOTNI16 = plan["tot_slots"], plan["tot_ni16"]
    PF_ACC = SLABP * EMB // 128
    PF_CH = GR * EMB // 128

    plan_s = build_sched(plan, cfg, n_layers)
    sched, sem_names = plan_s["sched"], plan_s["sems"]

    nc = bacc.Bacc("TRN2", detect_race_conditions=detect_races)
    x_in = nc.declare_dram_parameter("xfull", [N, EMB], FP, isOutput=False)
    xslab_in = nc.declare_dram_parameter("xslab", [SLABP, EMB], FP,
                                         isOutput=False)
    gidx_in = nc.declare_dram_parameter("gidx", [128, TOTNI16], I16,
                                        isOutput=False)
    gval_in = nc.declare_dram_parameter("gval", [128, TOTS], FP,
                                        isOutput=False)
    sidx_in = nc.declare_dram_parameter("sidx", [128, NU * (RANKS // 16)], I16,
                                        isOutput=False)
    out_ext = nc.declare_dram_parameter("out", [128, PF_ACC], FP,
                                        isOutput=True)

    xN = [nc.dram_tensor(f"xn{i}", [C * SLABP, EMB], FP) for i in range(2)]
    ybounce = nc.dram_tensor("ybounce", [SLABP, EMB], FP)
    agt = [nc.dram_tensor(f"agt{i}", [C * GR, EMB], FP, addr_space="Shared")
           for i in range(2)]

    from contextlib import ExitStack
    stack = ExitStack()
    with (
        stack,
        nc.Block() as block,
        nc.sbuf_tensor("tileA", [128, WMAX, EMB], FP) as tileA,
        nc.sbuf_tensor("tileB", [128, WMAX, EMB], FP) as tileB,
        nc.sbuf_tensor("gixA", [128, WMAX * 8], I16) as gixA,
        nc.sbuf_tensor("gixB", [128, WMAX * 8], I16) as gixB,
        nc.sbuf_tensor("gval_s", [128, TOTS], FP) as gval_s,
        nc.sbuf_tensor("sidx_s", [128, NU * (RANKS // 16)], I16) as sidx_s,
        nc.sbuf_tensor("yperm", [128, BL, EMB], FP) as yperm,
        nc.sbuf_tensor("zeros", [128, PF_CH], FP) as zeros,
        nc.sbuf_tensor("acc", [128, PF_ACC], FP) as acc,
        nc.sbuf_tensor("atmpA", [128, PF_CH], FP) as atmpA,
        nc.sbuf_tensor("atmpB", [128, PF_CH], FP) as atmpB,
    ):
        SEM = {s: stack.enter_context(nc.semaphore(s)) for s in sem_names}  # noqa: ANT232
        tiles = [tileA, tileB]
        gixs = [gixA, gixB]
        atmps = [atmpA, atmpB]

        def wv(e, op):
            for sname, val in op["waits"]:
                e.wait_ge(SEM[sname], val)

        def ap_of(spec):
            name, arg = spec
            if name == "gval_in":
                return gval_in[:, :]
            if name == "sidx_in":
                return sidx_in[:, :]
            if name == "gval_s":
                return gval_s[:, :]
            if name == "sidx_s":
                return sidx_s[:, :]
            if name == "xslab":
                # chunk-wise layout: acc[p, g*PF_CH+u] = xslab el g*GR*EMB + p*PF_CH + u
                return xslab_in[:, :].flatten().rearrange(
                    "(g p u) -> p g u", g=NG, p=128, u=PF_CH)
            if name == "acc":
                return acc[:, :]
            if name == "out":
                return out_ext[:, :]
            if name == "zeros":
                return zeros[:, :]
            if name == "ybounce_chunk":
                return ybounce[arg * GR:(arg + 1) * GR, :]
            if name == "atmp":
                return atmps[arg][:, :]
            if name == "gix":
                buf, w16 = arg
                return gixs[buf][:, :w16]
            if name == "gidx_slice":
                off, w16 = arg
                return gidx_in[:, off:off + w16]
            if name == "agt":
                return agt[arg][:, :]
            if name == "xn_stripe":
                Lc, gc = arg
                ap = xN[(Lc + 1) % 2][:, :].rearrange("(c r) e -> c r e", c=C)
                return ap[:, gc * GR:(gc + 1) * GR, :]
            raise KeyError(name)

        @block.gpsimd
        def _(e: bass.BassGpSimd):
            for op in sched:
                if op["eng"] != "g":
                    continue
                if op["kind"] == "lib":
                    e.load_library(mlp)
                    continue
                wv(e, op)
                if op["kind"] == "gather":
                    L, kk, ni, slots = (op["layer"], op["bucket"], op["ni"],
                                        op["slots"])
                    if L == 0:
                        table = x_in[kk * SLAB:(kk + 1) * SLAB, :]
                    else:
                        table = xN[L % 2][kk * SLABP:(kk + 1) * SLABP, :]
                    e.dma_gather(
                        tiles[op["tile"]][:, :slots, :], table,
                        gixs[op["tile"]][:, :ni // 16], ni, ni, EMB,
                        single_packet=False,
                    ).then_inc(SEM[op["inc_sem"]], 16)
                elif op["kind"] == "scatter":
                    uid, gg = op["uid"], op["g"]
                    sl = sidx_s[:, uid * (RANKS // 16):
                                (uid + 1) * (RANKS // 16)]
                    e.dma_scatter_add(
                        ybounce[gg * GR:(gg + 1) * GR, :], yperm[:, :, :],
                        sl, RANKS, GR, EMB, single_packet=False,
                    ).then_inc(SEM["ss"], 16)
                elif op["kind"] == "ag":
                    e.collective_compute(
                        "AllGather", mybir.AluOpType.bypass,
                        replica_groups=[list(range(C))],
                        ins=[ybounce[op["g"] * GR:(op["g"] + 1) * GR, :]],
                        outs=[agt[op["buf"]][:, :]],
                    ).then_inc(SEM[op["inc_sem"]], 1)

        @block.vector
        def _(e: bass.BassVectorEngine):
            for op in sched:
                if op["eng"] != "v":
                    continue
                wv(e, op)
                if op["kind"] == "memset":
                    e.memset(zeros[:, :], 0.0).then_inc(SEM["v"], 1)
                elif op["kind"] == "mult":
                    t = tiles[op["tile"]]
                    slots, voff = op["slots"], op["voff"]
                    vb = gval_s[:, voff:voff + slots].unsqueeze(-1) \
                        .broadcast_to((128, slots, EMB))
                    e.tensor_tensor(t[:, :slots, :], t[:, :slots, :], vb,
                                    mybir.AluOpType.mult).then_inc(SEM["v"], 1)
                elif op["kind"] == "reduce":
                    t = tiles[op["tile"]]
                    c0, D, nb, b0 = op["c0"], op["D"], op["nb"], op["b0"]
                    src = t[:, c0:c0 + nb * D, :].rearrange(
                        "p (nb d) e -> p nb e d", nb=nb, d=D)
                    e.tensor_reduce(yperm[:, b0:b0 + nb, :], src,
                                    mybir.AxisListType.X,
                                    mybir.AluOpType.add).then_inc(SEM["v"], 1)
                elif op["kind"] == "accadd":
                    sl = acc[:, op["g"] * PF_CH:(op["g"] + 1) * PF_CH]
                    e.tensor_tensor(sl, sl, atmps[op["buf"]][:, :],
                                    mybir.AluOpType.add).then_inc(SEM["v"], 1)
                elif op["kind"] == "scale":
                    e.tensor_scalar_mul(acc[:, :], acc[:, :],
                                        op["factor"]).then_inc(SEM["v"], 1)

        @block.sync
        def _(e):
            for op in sched:
                if op["eng"] != "x":
                    continue
                wv(e, op)
                if op["kind"] == "dma":
                    e.dma_start(out=ap_of(op["dst"]), in_=ap_of(op["src"])
                                ).then_inc(SEM[op["inc_sem"]], 16)

    nc.compile()
    return nc


# ---------------------------------------------------------------------------
# host entry
# ---------------------------------------------------------------------------

def _prep_inputs(user_emb, item_emb, adj_row, adj_col, adj_vals, cfg):
    C, SLAB, SLABP, EMB, N = (cfg["C"], cfg["SLAB"], cfg["SLABP"], cfg["EMB"],
                              cfg["N"])
    x = np.ascontiguousarray(
        np.concatenate([np.asarray(user_emb), np.asarray(item_emb)], axis=0)
        .astype(np.float32))
    plan = build_plan(adj_row, adj_col, adj_vals, cfg)
    in_maps = []
    for c in range(C):
        xs = np.zeros((SLABP, EMB), dtype=np.float32)
        xs[:SLAB] = x[c * SLAB:(c + 1) * SLAB]
        in_maps.append({
            "xfull": x,
            "xslab": xs,
            "gidx": np.ascontiguousarray(plan["gidx"][c]),
            "gval": np.ascontiguousarray(plan["gval"][c]),
            "sidx": np.ascontiguousarray(plan["sidx"][c]),
        })
    return plan, in_maps


def _unscramble(outs, cfg):
    C, NG, GR, SLAB, SLABP, EMB = (cfg["C"], cfg["NG"], cfg["GR"], cfg["SLAB"],
                                   cfg["SLABP"], cfg["EMB"])
    full = np.empty((cfg["N"], EMB), dtype=np.float32)
    for c in range(C):
        a = outs[c].reshape(128, NG, GR * EMB // 128)
        for g in range(NG):
            chunk = a[:, g, :].reshape(-1).reshape(GR, EMB)
            r0 = c * SLAB + g * GR
            nreal = min(GR, SLAB - g * GR)
            full[r0:r0 + nreal] = chunk[:nreal]
    return full


_last_exec_ns = None


def _install_ntff_hook():
    """The agent image's antenv lacks axon_hooks; synthesize it and register
    the ctypes NTFF profiling hook so trace=True yields exec_time_ns."""
    import types
    try:
        import antenv.axon_hooks  # noqa: F401
        return
    except ImportError:
        pass
    try:
        mod = types.ModuleType("antenv.axon_hooks")
        _h = [None]
        mod.get_axon_ntff_profile_hook = lambda: _h[0]
        mod.set_axon_ntff_profile_hook = lambda hk: _h.__setitem__(0, hk)
        sys.modules["antenv.axon_hooks"] = mod
        import antenv
        antenv.axon_hooks = mod
        if "/root/.axon_site" not in sys.path:
            sys.path.append("/root/.axon_site")
        from trn_agent_boot.trn_boot import _ntff_profile_via_ctypes
        hk = _ntff_profile_via_ctypes("/opt/axon/libaxon_pjrt.so")
        mod.set_axon_ntff_profile_hook(hk)
    except Exception as ex:  # degrade to no tracing
        print(f"[kernel] ntff hook install failed: {ex}", flush=True)


def kernel(user_emb, item_emb, adj_row, adj_col, adj_vals, n_layers,
           trace=True):
    global _last_exec_ns
    import time
    from concourse.bass_utils import run_bass_kernel_spmd

    t0 = time.time()
    _install_ntff_hook()
    n_layers = int(np.asarray(n_layers))
    cfg = make_cfg()
    plan, in_maps = _prep_inputs(user_emb, item_emb, adj_row, adj_col,
                                 adj_vals, cfg)
    t1 = time.time()
    nc = build_nc(plan, cfg, n_layers, detect_races=False)
    t2 = time.time()
    res = run_bass_kernel_spmd(nc, in_maps, list(range(cfg["C"])),
                               trace=trace)
    t3 = time.time()
    print(f"[kernel] prep {t1-t0:.1f}s build {t2-t1:.1f}s run {t3-t2:.1f}s",
          flush=True)
    _last_exec_ns = res.exec_time_ns
    outs = [res.results[c]["out"] for c in range(cfg["C"])]
    full = _unscramble(outs, cfg)
    return full[:USER_NUM], full[USER_NUM:]


# revision 24
# speedup vs baseline: 1.7039x; 1.7039x over previous
"""LightGCN-style GNN message passing (n_layers x SpMM + running mean) on 8 TRN2
NeuronCores.

Row-parallel SpMM (1D graph partition):
  - Core c owns dest-node slab [c*SLAB, (c+1)*SLAB).
  - Src nodes are split into C buckets == the C slabs, so every dma_gather index
    is slab-local (< 18750, fits the int16 index requirement).
  - Work unit = (dest group g of GR nodes, src bucket k).  Host builds a padded
    CSR per unit with nodes degree-sorted into blocks of 128 and a GLOBAL
    (core-max) block-degree profile so the SPMD graph is identical on all cores.
  - Per unit: per-edge dma_gather (f32 rows, 256 B) from the bucket table ->
    DVE multiply by vals -> DVE strided segment-reduce per equal-degree block
    run -> dma_scatter_add merges the bucket partial into the HBM y slab.
  - Per dest group g one small AllGather broadcasts all cores' group-g rows;
    a local re-stripe DMA lays them out slab-contiguously as next layer's
    gather tables.  The C small AllGathers per layer pipeline behind compute.
  - acc (running sum over layer outputs) lives in SBUF f32; final scale by
    1/(n_layers+1); host unscrambles the partition-major output layout.
"""

import sys
import numpy as np

if "/opt/trn_rl_repo" not in sys.path:
    sys.path.insert(0, "/opt/trn_rl_repo")

USER_NUM = 100000
ITEM_NUM = 50000


def make_cfg(n_nodes=150000, emb=64, n_cores=8, n_groups=8, wmax=32, nb=8):
    slab = n_nodes // n_cores
    assert slab * n_cores == n_nodes
    gr = -(-slab // n_groups)
    ranks = -(-gr // 128) * 128
    return dict(N=n_nodes, EMB=emb, C=n_cores, NG=n_groups, GR=gr, RANKS=ranks,
                BL=ranks // 128, SLAB=slab, SLABP=gr * n_groups, WMAX=wmax,
                NB=nb)


# ---------------------------------------------------------------------------
# host-side plan
# ---------------------------------------------------------------------------

def build_plan(rows, cols, vals, cfg):
    C, NG, GR, RANKS, BL, SLAB, WMAX = (
        cfg["C"], cfg["NG"], cfg["GR"], cfg["RANKS"], cfg["BL"], cfg["SLAB"],
        cfg["WMAX"])
    NU = NG * C

    rows = np.asarray(rows, dtype=np.int64)
    cols = np.asarray(cols, dtype=np.int64)
    vals = np.asarray(vals, dtype=np.float32)
    c = rows // SLAB
    loc = rows - c * SLAB
    g = np.minimum(loc // GR, NG - 1)
    grow = loc - g * GR
    k = cols // SLAB
    colloc = (cols - k * SLAB).astype(np.int16)

    cu = (c * NU + g * C + k).astype(np.int64)
    nkey = cu * GR + grow

    deg = np.bincount(nkey, minlength=C * NU * GR).reshape(C * NU, GR)
    order = np.argsort(-deg, axis=1, kind="stable")          # rank -> node id
    rank_of = np.empty_like(order)
    np.put_along_axis(rank_of, order,
                      np.broadcast_to(np.arange(GR), (C * NU, GR)), axis=1)

    ds = np.zeros((C * NU, RANKS), dtype=np.int64)
    ds[:, :GR] = np.take_along_axis(deg, order, axis=1)
    bmax = ds.reshape(C * NU, BL, 128).max(axis=2)
    Dprof = bmax.reshape(C, NU, BL).max(axis=0)              # [NU, BL]
    assert Dprof.max() <= WMAX, f"block degree {Dprof.max()} > WMAX {WMAX}"

    # tile packing + global slot offsets per block
    glob_blk_slot = np.zeros((NU, BL), dtype=np.int64)
    units = []
    tot_slots = 0
    for u in range(NU):
        gg, kk = divmod(u, C)
        nbl = int((Dprof[u] > 0).sum())      # zero-D blocks only at the end
        tiles = []
        b = 0
        while b < nbl:
            slots = 0
            runs = []
            tile_off = tot_slots
            b0 = b
            while b < nbl and slots + int(Dprof[u, b]) <= WMAX:
                D = int(Dprof[u, b])
                glob_blk_slot[u, b] = tile_off + slots
                if runs and runs[-1][0] == D:
                    runs[-1][1] += 1
                else:
                    runs.append([D, 1])
                slots += D
                b += 1
            tiles.append(dict(slots=slots, runs=[tuple(r) for r in runs],
                              b0=b0, slot_off=tile_off))
            tot_slots += slots
        units.append(dict(g=gg, k=kk, uid=u, tiles=tiles))

    # within-node edge counter j
    es = np.argsort(nkey, kind="stable")
    nk_s = nkey[es]
    first = np.r_[0, np.flatnonzero(np.diff(nk_s)) + 1]
    starts = np.zeros(len(nk_s), dtype=np.int64)
    starts[first] = first
    starts = np.maximum.accumulate(starts)
    j = np.empty_like(starts)
    j[es] = np.arange(len(nk_s)) - starts

    r_e = rank_of[cu, grow]
    u_e = cu % NU
    slot_e = glob_blk_slot[u_e, r_e // 128] + j
    pos = slot_e * 128 + (r_e % 128)

    gidx_lin = np.zeros((C, tot_slots * 128), dtype=np.int16)
    gval_lin = np.zeros((C, tot_slots * 128), dtype=np.float32)
    gidx_lin[c, pos] = colloc
    gval_lin[c, pos] = vals

    gval_w = np.ascontiguousarray(
        gval_lin.reshape(C, tot_slots, 128).transpose(0, 2, 1))

    # wrap gather idx per tile -> [C, 128, tot_slots*8]
    gidx_w = np.zeros((C, 128, tot_slots * 8), dtype=np.int16)
    off16 = 0
    for u in range(NU):
        for t in units[u]["tiles"]:
            ni = t["slots"] * 128
            a = t["slot_off"] * 128
            seg = gidx_lin[:, a:a + ni].reshape(C, ni // 16, 16)
            gidx_w[:, :16, off16:off16 + ni // 16] = seg.transpose(0, 2, 1)
            t["ni_off16"] = off16
            off16 += ni // 16
    gidx_w[:, 16:, :] = np.tile(gidx_w[:, :16, :], (1, 7, 1))

    # scatter idx per unit
    sidx_lin = np.full((C, NU, RANKS), -1, dtype=np.int16)
    sidx_lin[:, :, :GR] = order.reshape(C, NU, GR).astype(np.int16)
    seg = sidx_lin.reshape(C, NU, RANKS // 16, 16).transpose(0, 1, 3, 2)
    sidx_w = np.broadcast_to(seg[:, :, None, :, :],
                             (C, NU, 8, 16, RANKS // 16))
    sidx_w = np.ascontiguousarray(
        sidx_w.reshape(C, NU, 128, RANKS // 16).transpose(0, 2, 1, 3)
        .reshape(C, 128, NU * (RANKS // 16)))

    return dict(units=units, tot_slots=tot_slots, tot_ni16=off16,
                gidx=gidx_w, gval=gval_w, sidx=sidx_w)


# ---------------------------------------------------------------------------
# schedule (engine-op list with cumulative semaphore targets)
# ---------------------------------------------------------------------------

def build_sched(plan, cfg, n_layers):
    """Emit a per-engine op list.  Semaphore discipline: every wait value is
    the total-so-far of its semaphore, and any two DMAs sharing a semaphore
    are serialized by a consumer dependency, so thresholds are race-free.
    Gathers rotate over NB buffers/semaphores and 4 SWDGE queues for deep
    desc-gen pipelining."""
    C, NG, NB = cfg["C"], cfg["NG"], cfg["NB"]
    units = plan["units"]
    tiles_flat = []
    for u in units:
        for ti, t in enumerate(u["tiles"]):
            tiles_flat.append((u, ti, t))
    TPL = len(tiles_flat)
    NT = TPL * n_layers

    SEMS = (["ss", "c0", "c1", "is", "a0", "a1", "r0", "r1", "v", "os"]
            + [f"z{g}" for g in range(NG)]
            + [f"g{i}" for i in range(NB)] + [f"sx{i}" for i in range(NB)])
    cnt = {s: 0 for s in SEMS}
    sched = []

    def emit(eng, kind, waits=(), inc=None, **kw):
        w = {}
        for sname, val in waits:
            w[sname] = max(w.get(sname, 0), val)
        sched.append(dict(eng=eng, kind=kind,
                          waits=[(s_, v_) for s_, v_ in w.items() if v_ > 0],
                          inc_sem=inc[0] if inc else None, **kw))
        if inc:
            cnt[inc[0]] += inc[1]

    v_after_tile = {}
    g_after_tile = {}
    sx_after_stream = {}
    s_after_unit = {}
    s_after_ucnt = {}
    r_after = {}
    c_after_ag = {}
    v_after_accadd = {}
    pending = []

    emit("g", "lib")
    emit("x", "dma", dst=("gval_s", None), src=("gval_in", None), inc=("is", 16))
    emit("x", "dma", dst=("sidx_s", None), src=("sidx_in", None), inc=("is", 16))
    emit("x", "dma", dst=("acc", None), src=("xslab", None), inc=("is", 16))
    emit("v", "memset", inc=("v", 1))
    zero_v = cnt["v"]
    if n_layers > 0:
        for gg in range(NG):
            emit("x", "dma", dst=("ybounce_chunk", gg), src=("zeros", None),
                 waits=[("v", zero_v)], inc=(f"z{gg}", 16))

    def stream(tg):
        if tg >= NT:
            return
        u, ti, t = tiles_flat[tg % TPL]
        p = tg % NB
        waits = []
        if tg >= NB:
            waits.append((f"g{p}", g_after_tile[tg - NB]))
        emit("x", "dma", dst=("gix", (p, t["slots"] * 8)),
             src=("gidx_slice", (t["ni_off16"], t["slots"] * 8)),
             waits=waits, inc=(f"sx{p}", 16))
        sx_after_stream[tg] = cnt[f"sx{p}"]

    def flush(cur_key):
        while pending and pending[0][0] <= cur_key - 2:
            _, fn = pending.pop(0)
            fn()

    for i in range(NB):
        stream(i)

    ucnt = 0   # global unit counter (for yperm parity)
    for L in range(n_layers):
        flush(L * NG + 1)
        for gt, (u, ti, t) in enumerate(tiles_flat):
            tg = L * TPL + gt
            p = tg % NB
            uid, gg, kk = u["uid"], u["g"], u["k"]
            flush(L * NG + gg)

            waits = [(f"sx{p}", sx_after_stream[tg])]
            if tg >= NB:
                waits.append(("v", v_after_tile[tg - NB]))
            if L >= 1:
                waits.append(("r0", r_after.get(("r0", L - 1), 0)))
                waits.append(("r1", r_after.get(("r1", L - 1), 0)))
            emit("g", "gather", tile=p, slots=t["slots"], layer=L,
                 bucket=kk, ni=t["slots"] * 128, queue=tg % 4, waits=waits,
                 inc=(f"g{p}", 16))
            g_after_tile[tg] = cnt[f"g{p}"]

            stream(tg + NB)

            waits = [(f"g{p}", g_after_tile[tg])]
            if tg == 0:
                waits.append(("is", 48))
            emit("v", "mult", tile=p, slots=t["slots"], voff=t["slot_off"],
                 waits=waits, inc=("v", 1))
            first_red = (ti == 0)
            c0 = 0
            b = t["b0"]
            for (D, nb) in t["runs"]:
                w = []
                if first_red and ucnt >= 2:
                    w = [("ss", s_after_ucnt[ucnt - 2])]
                first_red = False
                emit("v", "reduce", tile=p, c0=c0, D=D, nb=nb, b0=b,
                     yp=ucnt % 2, waits=w, inc=("v", 1))
                c0 += D * nb
                b += nb
            v_after_tile[tg] = cnt["v"]

            if ti == len(u["tiles"]) - 1:
                waits = [("v", v_after_tile[tg]),
                         (f"z{gg}", 16 * (L + 1)),
                         ("ss", cnt["ss"])]
                if cnt["ss"] == 0:
                    waits.append(("is", 48))
                emit("g", "scatter", uid=uid, g=gg, yp=ucnt % 2,
                     queue=0, waits=waits, inc=("ss", 16))
                s_after_unit[(L, uid)] = cnt["ss"]
                s_after_ucnt[ucnt] = cnt["ss"]
                ucnt += 1

                if kk == C - 1:
                    if L < n_layers - 1:
                        agk = L * NG + gg
                        waits = [("ss", cnt["ss"])]
                        if agk >= 2:
                            waits.append((f"r{agk % 2}",
                                          16 * (agk // 2)))
                        emit("g", "ag", g=gg, buf=agk % 2, waits=waits,
                             inc=(f"c{agk % 2}", 1))
                        c_after_ag[(L, gg)] = cnt[f"c{agk % 2}"]

                    def mk(Lc, gc):
                        def fn():
                            pa = (Lc * NG + gc) % 2
                            w = [("ss", s_after_unit[(Lc, gc * C + C - 1)])]
                            prev = (Lc * NG + gc) - 2
                            if prev >= 0:
                                Lp, gp = divmod(prev, NG)
                                if (Lp, gp) in v_after_accadd:
                                    w.append(("v", v_after_accadd[(Lp, gp)]))
                            emit("x", "dma", dst=("atmp", pa),
                                 src=("ybounce_chunk", gc), waits=w,
                                 inc=(f"a{pa}", 16))
                            emit("v", "accadd", g=gc, buf=pa,
                                 waits=[(f"a{pa}", cnt[f"a{pa}"])],
                                 inc=("v", 1))
                            v_after_accadd[(Lc, gc)] = cnt["v"]
                            if Lc < n_layers - 1:
                                agk = Lc * NG + gc
                                emit("x", "dma", dst=("xn_stripe", (Lc, gc)),
                                     src=("agt", agk % 2),
                                     waits=[(f"c{agk % 2}",
                                             c_after_ag[(Lc, gc)])],
                                     inc=(f"r{agk % 2}", 16))
                                r_after[(f"r{agk % 2}", Lc)] = \
                                    cnt[f"r{agk % 2}"]
                                emit("x", "dma", dst=("ybounce_chunk", gc),
                                     src=("zeros", None),
                                     waits=[(f"a{pa}", cnt[f"a{pa}"])],
                                     inc=(f"z{gc}", 16))
                        return fn

                    pending.append((L * NG + gg, mk(L, gg)))

    flush(10 ** 9)
    emit("v", "scale", factor=1.0 / float(n_layers + 1), inc=("v", 1))
    emit("x", "dma", dst=("out", None), src=("acc", None),
         waits=[("v", cnt["v"])], inc=("os", 16))
    emit("x", "wait", waits=[("os", cnt["os"])])
    emit("g", "wait", waits=[("ss", cnt["ss"])]
         + [(f"g{i}", cnt[f"g{i}"]) for i in range(NB)])
    return dict(sched=sched, sems=SEMS)


# ---------------------------------------------------------------------------
# bass graph
# ---------------------------------------------------------------------------

def build_nc(plan, cfg, n_layers, detect_races=True):
    import concourse.bass as bass
    import concourse.bacc as bacc
    import concourse.mybir as mybir
    from concourse.library_config import mlp

    C, NG, GR, RANKS, BL, SLAB, SLABP, EMB, WMAX, N = (
        cfg["C"], cfg["NG"], cfg["GR"], cfg["RANKS"], cfg["BL"], cfg["SLAB"],
        cfg["SLABP"], cfg["EMB"], cfg["WMAX"], cfg["N"])
    NU = NG * C
    FP, I16 = mybir.dt.float32, mybir.dt.int16
    TOTS, TOTNI16 = plan["tot_slots"], plan["tot_ni16"]
    PF_ACC = SLABP * EMB // 128
    PF_CH = GR * EMB // 128

    plan_s = build_sched(plan, cfg, n_layers)
    sched, sem_names = plan_s["sched"], plan_s["sems"]

    nc = bacc.Bacc("TRN2", detect_race_conditions=detect_races,
                   num_swdge_queues=4)
    x_in = nc.declare_dram_parameter("xfull", [N, EMB], FP, isOutput=False)
    xslab_in = nc.declare_dram_parameter("xslab", [SLABP, EMB], FP,
                                         isOutput=False)
    gidx_in = nc.declare_dram_parameter("gidx", [128, TOTNI16], I16,
                                        isOutput=False)
    gval_in = nc.declare_dram_parameter("gval", [128, TOTS], FP,
                                        isOutput=False)
    sidx_in = nc.declare_dram_parameter("sidx", [128, NU * (RANKS // 16)], I16,
                                        isOutput=False)
    out_ext = nc.declare_dram_parameter("out", [128, PF_ACC], FP,
                                        isOutput=True)

    xN = [nc.dram_tensor(f"xn{i}", [C * SLABP, EMB], FP) for i in range(2)]
    ybounce = nc.dram_tensor("ybounce", [SLABP, EMB], FP)
    agt = [nc.dram_tensor(f"agt{i}", [C * GR, EMB], FP, addr_space="Shared")
           for i in range(2)]

    from contextlib import ExitStack
    NB = cfg["NB"]
    stack = ExitStack()
    with (
        stack,
        nc.Block() as block,
    ):
        tiles = [stack.enter_context(  # noqa: ANT232
            nc.sbuf_tensor(f"tile{i}", [128, WMAX, EMB], FP))
            for i in range(NB)]
        gixs = [stack.enter_context(  # noqa: ANT232
            nc.sbuf_tensor(f"gix{i}", [128, WMAX * 8], I16))
            for i in range(NB)]
        gval_s = stack.enter_context(nc.sbuf_tensor("gval_s", [128, TOTS], FP))
        sidx_s = stack.enter_context(
            nc.sbuf_tensor("sidx_s", [128, NU * (RANKS // 16)], I16))
        yperms = [stack.enter_context(  # noqa: ANT232
            nc.sbuf_tensor(f"yperm{i}", [128, BL, EMB], FP)) for i in range(2)]
        zeros = stack.enter_context(nc.sbuf_tensor("zeros", [128, PF_CH], FP))
        acc = stack.enter_context(nc.sbuf_tensor("acc", [128, PF_ACC], FP))
        atmps = [stack.enter_context(  # noqa: ANT232
            nc.sbuf_tensor(f"atmp{i}", [128, PF_CH], FP)) for i in range(2)]
        SEM = {s: stack.enter_context(nc.semaphore(s)) for s in sem_names}  # noqa: ANT232

        def wv(e, op):
            for sname, val in op["waits"]:
                e.wait_ge(SEM[sname], val)

        def ap_of(spec):
            name, arg = spec
            if name == "gval_in":
                return gval_in[:, :]
            if name == "sidx_in":
                return sidx_in[:, :]
            if name == "gval_s":
                return gval_s[:, :]
            if name == "sidx_s":
                return sidx_s[:, :]
            if name == "xslab":
                # chunk-wise layout: acc[p, g*PF_CH+u] = xslab el g*GR*EMB + p*PF_CH + u
                return xslab_in[:, :].flatten().rearrange(
                    "(g p u) -> p g u", g=NG, p=128, u=PF_CH)
            if name == "acc":
                return acc[:, :]
            if name == "out":
                return out_ext[:, :]
            if name == "zeros":
                return zeros[:, :]
            if name == "ybounce_chunk":
                return ybounce[arg * GR:(arg + 1) * GR, :]
            if name == "atmp":
                return atmps[arg][:, :]
            if name == "gix":
                buf, w16 = arg
                return gixs[buf][:, :w16]
            if name == "gidx_slice":
                off, w16 = arg
                return gidx_in[:, off:off + w16]
            if name == "agt":
                return agt[arg][:, :]
            if name == "xn_stripe":
                Lc, gc = arg
                ap = xN[(Lc + 1) % 2][:, :].rearrange("(c r) e -> c r e", c=C)
                return ap[:, gc * GR:(gc + 1) * GR, :]
            raise KeyError(name)

        @block.gpsimd
        def _(e: bass.BassGpSimd):
            for op in sched:
                if op["eng"] != "g":
                    continue
                if op["kind"] == "lib":
                    e.load_library(mlp)
                    continue
                wv(e, op)
                if op["kind"] == "gather":
                    L, kk, ni, slots = (op["layer"], op["bucket"], op["ni"],
                                        op["slots"])
                    if L == 0:
                        table = x_in[kk * SLAB:(kk + 1) * SLAB, :]
                    else:
                        table = xN[L % 2][kk * SLABP:(kk + 1) * SLABP, :]
                    e.dma_gather(
                        tiles[op["tile"]][:, :slots, :], table,
                        gixs[op["tile"]][:, :ni // 16], ni, ni, EMB,
                        single_packet=False, queue_num=op["queue"],
                    ).then_inc(SEM[op["inc_sem"]], 16)
                elif op["kind"] == "scatter":
                    uid, gg = op["uid"], op["g"]
                    sl = sidx_s[:, uid * (RANKS // 16):
                                (uid + 1) * (RANKS // 16)]
                    e.dma_scatter_add(
                        ybounce[gg * GR:(gg + 1) * GR, :],
                        yperms[op["yp"]][:, :, :],
                        sl, RANKS, GR, EMB, single_packet=False,
                        queue_num=op["queue"],
                    ).then_inc(SEM["ss"], 16)
                elif op["kind"] == "ag":
                    e.collective_compute(
                        "AllGather", mybir.AluOpType.bypass,
                        replica_groups=[list(range(C))],
                        ins=[ybounce[op["g"] * GR:(op["g"] + 1) * GR, :]],
                        outs=[agt[op["buf"]][:, :]],
                    ).then_inc(SEM[op["inc_sem"]], 1)

        @block.vector
        def _(e: bass.BassVectorEngine):
            for op in sched:
                if op["eng"] != "v":
                    continue
                wv(e, op)
                if op["kind"] == "memset":
                    e.memset(zeros[:, :], 0.0).then_inc(SEM["v"], 1)
                elif op["kind"] == "mult":
                    t = tiles[op["tile"]]
                    slots, voff = op["slots"], op["voff"]
                    vb = gval_s[:, voff:voff + slots].unsqueeze(-1) \
                        .broadcast_to((128, slots, EMB))
                    e.tensor_tensor(t[:, :slots, :], t[:, :slots, :], vb,
                                    mybir.AluOpType.mult).then_inc(SEM["v"], 1)
                elif op["kind"] == "reduce":
                    t = tiles[op["tile"]]
                    c0, D, nb, b0 = op["c0"], op["D"], op["nb"], op["b0"]
                    src = t[:, c0:c0 + nb * D, :].rearrange(
                        "p (nb d) e -> p nb e d", nb=nb, d=D)
                    e.tensor_reduce(yperms[op["yp"]][:, b0:b0 + nb, :], src,
                                    mybir.AxisListType.X,
                                    mybir.AluOpType.add).then_inc(SEM["v"], 1)
                elif op["kind"] == "accadd":
                    sl = acc[:, op["g"] * PF_CH:(op["g"] + 1) * PF_CH]
                    e.tensor_tensor(sl, sl, atmps[op["buf"]][:, :],
                                    mybir.AluOpType.add).then_inc(SEM["v"], 1)
                elif op["kind"] == "scale":
                    e.tensor_scalar_mul(acc[:, :], acc[:, :],
                                        op["factor"]).then_inc(SEM["v"], 1)

        @block.sync
        def _(e):
            for op in sched:
                if op["eng"] != "x":
                    continue
                wv(e, op)
                if op["kind"] == "dma":
                    e.dma_start(out=ap_of(op["dst"]), in_=ap_of(op["src"])
                                ).then_inc(SEM[op["inc_sem"]], 16)

    nc.compile()
    return nc


# ---------------------------------------------------------------------------
# host entry
# ---------------------------------------------------------------------------

def _prep_inputs(user_emb, item_emb, adj_row, adj_col, adj_vals, cfg):
    C, SLAB, SLABP, EMB, N = (cfg["C"], cfg["SLAB"], cfg["SLABP"], cfg["EMB"],
                              cfg["N"])
    x = np.ascontiguousarray(
        np.concatenate([np.asarray(user_emb), np.asarray(item_emb)], axis=0)
        .astype(np.float32))
    plan = build_plan(adj_row, adj_col, adj_vals, cfg)
    in_maps = []
    for c in range(C):
        xs = np.zeros((SLABP, EMB), dtype=np.float32)
        xs[:SLAB] = x[c * SLAB:(c + 1) * SLAB]
        in_maps.append({
            "xfull": x,
            "xslab": xs,
            "gidx": np.ascontiguousarray(plan["gidx"][c]),
            "gval": np.ascontiguousarray(plan["gval"][c]),
            "sidx": np.ascontiguousarray(plan["sidx"][c]),
        })
    return plan, in_maps


def _unscramble(outs, cfg):
    C, NG, GR, SLAB, SLABP, EMB = (cfg["C"], cfg["NG"], cfg["GR"], cfg["SLAB"],
                                   cfg["SLABP"], cfg["EMB"])
    full = np.empty((cfg["N"], EMB), dtype=np.float32)
    for c in range(C):
        a = outs[c].reshape(128, NG, GR * EMB // 128)
        for g in range(NG):
            chunk = a[:, g, :].reshape(-1).reshape(GR, EMB)
            r0 = c * SLAB + g * GR
            nreal = min(GR, SLAB - g * GR)
            full[r0:r0 + nreal] = chunk[:nreal]
    return full


_last_exec_ns = None


def _install_ntff_hook():
    """The agent image's antenv lacks axon_hooks; synthesize it and register
    the ctypes NTFF profiling hook so trace=True yields exec_time_ns."""
    import types
    try:
        import antenv.axon_hooks  # noqa: F401
        return
    except ImportError:
        pass
    try:
        mod = types.ModuleType("antenv.axon_hooks")
        _h = [None]
        mod.get_axon_ntff_profile_hook = lambda: _h[0]
        mod.set_axon_ntff_profile_hook = lambda hk: _h.__setitem__(0, hk)
        sys.modules["antenv.axon_hooks"] = mod
        import antenv
        antenv.axon_hooks = mod
        if "/root/.axon_site" not in sys.path:
            sys.path.append("/root/.axon_site")
        from trn_agent_boot.trn_boot import _ntff_profile_via_ctypes
        hk = _ntff_profile_via_ctypes("/opt/axon/libaxon_pjrt.so")
        mod.set_axon_ntff_profile_hook(hk)
    except Exception as ex:  # degrade to no tracing
        print(f"[kernel] ntff hook install failed: {ex}", flush=True)


def kernel(user_emb, item_emb, adj_row, adj_col, adj_vals, n_layers,
           trace=True):
    global _last_exec_ns
    import time
    from concourse.bass_utils import run_bass_kernel_spmd

    t0 = time.time()
    _install_ntff_hook()
    n_layers = int(np.asarray(n_layers))
    cfg = make_cfg()
    plan, in_maps = _prep_inputs(user_emb, item_emb, adj_row, adj_col,
                                 adj_vals, cfg)
    t1 = time.time()
    nc = build_nc(plan, cfg, n_layers, detect_races=False)
    t2 = time.time()
    res = run_bass_kernel_spmd(nc, in_maps, list(range(cfg["C"])),
                               trace=trace)
    t3 = time.time()
    print(f"[kernel] prep {t1-t0:.1f}s build {t2-t1:.1f}s run {t3-t2:.1f}s",
          flush=True)
    _last_exec_ns = res.exec_time_ns
    outs = [res.results[c]["out"] for c in range(cfg["C"])]
    full = _unscramble(outs, cfg)
    return full[:USER_NUM], full[USER_NUM:]


# revision 32
# speedup vs baseline: 1.9564x; 1.1482x over previous
"""LightGCN-style GNN message passing (n_layers x SpMM + running mean) on 8 TRN2
NeuronCores.

Row-parallel SpMM (1D graph partition):
  - Core c owns dest-node slab [c*SLAB, (c+1)*SLAB).
  - Src nodes are split into C buckets == the C slabs, so every dma_gather index
    is slab-local (< 18750, fits the int16 index requirement).
  - Work unit = (dest group g of GR nodes, src bucket k).  Host builds a padded
    CSR per unit with nodes degree-sorted into blocks of 128 and a GLOBAL
    (core-max) block-degree profile so the SPMD graph is identical on all cores.
  - Per unit: per-edge dma_gather (f32 rows, 256 B) from the bucket table ->
    DVE multiply by vals -> DVE strided segment-reduce per equal-degree block
    run -> dma_scatter_add merges the bucket partial into the HBM y slab.
  - Per dest group g one small AllGather broadcasts all cores' group-g rows;
    a local re-stripe DMA lays them out slab-contiguously as next layer's
    gather tables.  The C small AllGathers per layer pipeline behind compute.
  - acc (running sum over layer outputs) lives in SBUF f32; final scale by
    1/(n_layers+1); host unscrambles the partition-major output layout.
"""

import sys
import numpy as np

if "/opt/trn_rl_repo" not in sys.path:
    sys.path.insert(0, "/opt/trn_rl_repo")

USER_NUM = 100000
ITEM_NUM = 50000


def make_cfg(n_nodes=150000, emb=64, n_cores=8, n_groups=8, wmax=32, nb=8):
    slab = n_nodes // n_cores
    assert slab * n_cores == n_nodes
    gr = -(-slab // n_groups)
    ranks = -(-gr // 128) * 128
    return dict(N=n_nodes, EMB=emb, C=n_cores, NG=n_groups, GR=gr, RANKS=ranks,
                BL=ranks // 128, SLAB=slab, SLABP=gr * n_groups, WMAX=wmax,
                NB=nb)


# ---------------------------------------------------------------------------
# host-side plan
# ---------------------------------------------------------------------------

def build_plan(rows, cols, vals, cfg):
    C, NG, GR, RANKS, BL, SLAB, WMAX = (
        cfg["C"], cfg["NG"], cfg["GR"], cfg["RANKS"], cfg["BL"], cfg["SLAB"],
        cfg["WMAX"])
    NU = NG * C

    rows = np.asarray(rows, dtype=np.int64)
    cols = np.asarray(cols, dtype=np.int64)
    vals = np.asarray(vals, dtype=np.float32)
    c = rows // SLAB
    loc = rows - c * SLAB
    g = np.minimum(loc // GR, NG - 1)
    grow = loc - g * GR
    k = cols // SLAB
    colloc = (cols - k * SLAB).astype(np.int16)

    cu = (c * NU + g * C + k).astype(np.int64)
    nkey = cu * GR + grow

    deg = np.bincount(nkey, minlength=C * NU * GR).reshape(C * NU, GR)
    order = np.argsort(-deg, axis=1, kind="stable")          # rank -> node id
    rank_of = np.empty_like(order)
    np.put_along_axis(rank_of, order,
                      np.broadcast_to(np.arange(GR), (C * NU, GR)), axis=1)

    ds = np.zeros((C * NU, RANKS), dtype=np.int64)
    ds[:, :GR] = np.take_along_axis(deg, order, axis=1)
    bmax = ds.reshape(C * NU, BL, 128).max(axis=2)
    Dprof = bmax.reshape(C, NU, BL).max(axis=0)              # [NU, BL]
    assert Dprof.max() <= WMAX, f"block degree {Dprof.max()} > WMAX {WMAX}"

    # tile packing + global slot offsets per block
    glob_blk_slot = np.zeros((NU, BL), dtype=np.int64)
    units = []
    tot_slots = 0
    for u in range(NU):
        gg, kk = divmod(u, C)
        nbl = int((Dprof[u] > 0).sum())      # zero-D blocks only at the end
        tiles = []
        b = 0
        while b < nbl:
            slots = 0
            runs = []
            tile_off = tot_slots
            b0 = b
            while b < nbl and slots + int(Dprof[u, b]) <= WMAX:
                D = int(Dprof[u, b])
                glob_blk_slot[u, b] = tile_off + slots
                if runs and runs[-1][0] == D:
                    runs[-1][1] += 1
                else:
                    runs.append([D, 1])
                slots += D
                b += 1
            tiles.append(dict(slots=slots, runs=[tuple(r) for r in runs],
                              b0=b0, slot_off=tile_off))
            tot_slots += slots
        units.append(dict(g=gg, k=kk, uid=u, tiles=tiles))

    # within-node edge counter j
    es = np.argsort(nkey, kind="stable")
    nk_s = nkey[es]
    first = np.r_[0, np.flatnonzero(np.diff(nk_s)) + 1]
    starts = np.zeros(len(nk_s), dtype=np.int64)
    starts[first] = first
    starts = np.maximum.accumulate(starts)
    j = np.empty_like(starts)
    j[es] = np.arange(len(nk_s)) - starts

    r_e = rank_of[cu, grow]
    u_e = cu % NU
    slot_e = glob_blk_slot[u_e, r_e // 128] + j
    pos = slot_e * 128 + (r_e % 128)

    gidx_lin = np.zeros((C, tot_slots * 128), dtype=np.int16)
    gval_lin = np.zeros((C, tot_slots * 128), dtype=np.float32)
    gidx_lin[c, pos] = colloc
    gval_lin[c, pos] = vals

    gval_w = np.ascontiguousarray(
        gval_lin.reshape(C, tot_slots, 128).transpose(0, 2, 1))

    # wrap gather idx per tile -> [C, 128, tot_slots*8]
    gidx_w = np.zeros((C, 128, tot_slots * 8), dtype=np.int16)
    off16 = 0
    for u in range(NU):
        for t in units[u]["tiles"]:
            ni = t["slots"] * 128
            a = t["slot_off"] * 128
            seg = gidx_lin[:, a:a + ni].reshape(C, ni // 16, 16)
            gidx_w[:, :16, off16:off16 + ni // 16] = seg.transpose(0, 2, 1)
            t["ni_off16"] = off16
            off16 += ni // 16
    gidx_w[:, 16:, :] = np.tile(gidx_w[:, :16, :], (1, 7, 1))

    # scatter idx per unit
    sidx_lin = np.full((C, NU, RANKS), -1, dtype=np.int16)
    sidx_lin[:, :, :GR] = order.reshape(C, NU, GR).astype(np.int16)
    seg = sidx_lin.reshape(C, NU, RANKS // 16, 16).transpose(0, 1, 3, 2)
    sidx_w = np.broadcast_to(seg[:, :, None, :, :],
                             (C, NU, 8, 16, RANKS // 16))
    sidx_w = np.ascontiguousarray(
        sidx_w.reshape(C, NU, 128, RANKS // 16).transpose(0, 2, 1, 3)
        .reshape(C, 128, NU * (RANKS // 16)))

    return dict(units=units, tot_slots=tot_slots, tot_ni16=off16,
                gidx=gidx_w, gval=gval_w, sidx=sidx_w)


# ---------------------------------------------------------------------------
# schedule (engine-op list with cumulative semaphore targets)
# ---------------------------------------------------------------------------

def build_sched(plan, cfg, n_layers):
    """Engine-op schedule.  Invariants: every semaphore wait value is a
    total-so-far on that sem at emit time, and any two DMAs sharing a sem are
    ordered by a consumer dependency (or waited as a full-burst total), so
    thresholds are race-free.  Gathers rotate over NB buffers/sems and SWDGE
    queues 0-3; scatters write disjoint per-bucket HBM bank regions
    (race-free) on queues 0/1 with 2 rotating sems.  Bank partials are summed
    on DVE per dest group (boundaryA, lag 2 groups), the AllGather is issued
    on gpsimd during group g+2, and restripe+bank-rezero run at lag 3
    (boundaryB), so the gpsimd engine never stalls mid-pipeline."""
    C, NG, NB = cfg["C"], cfg["NG"], cfg["NB"]
    units = plan["units"]
    tiles_flat = []
    for u in units:
        for ti, t in enumerate(u["tiles"]):
            tiles_flat.append((u, ti, t))
    TPL = len(tiles_flat)
    NT = TPL * n_layers

    SEMS = (["ss0", "ss1", "c0", "c1", "is", "a0", "a1", "r0", "r1", "v",
             "os", "ys0", "ys1"]
            + [f"z{g}" for g in range(NG)]
            + [f"g{i}" for i in range(NB)] + [f"sx{i}" for i in range(NB)])
    cnt = {s: 0 for s in SEMS}
    sched = []

    def emit(eng, kind, waits=(), inc=None, **kw):
        w = {}
        for sname, val in waits:
            w[sname] = max(w.get(sname, 0), val)
        sched.append(dict(eng=eng, kind=kind,
                          waits=[(s_, v_) for s_, v_ in w.items() if v_ > 0],
                          inc_sem=inc[0] if inc else None, **kw))
        if inc:
            cnt[inc[0]] += inc[1]

    v_after_tile = {}
    g_after_tile = {}
    sx_after_stream = {}
    s_after_ucnt = {}
    v_after_unit = {}
    r_after = {}
    c_after_ag = {}
    v_after_accadd = {}
    ys_after = {}
    pendA = []      # ysum/accadd thunks, flushed at lag 2 (sync+vector)
    pendB = []      # restripe + bank-rezero thunks, lag 3 (sync)
    sdefer = []     # deferred gpsimd scatters keyed by ucnt
    agdefer = []    # deferred gpsimd AGs keyed by ucnt

    emit("g", "lib")
    emit("x", "dma", dst=("gval_s", None), src=("gval_in", None), inc=("is", 16))
    emit("x", "dma", dst=("sidx_s", None), src=("sidx_in", None), inc=("is", 16))
    emit("x", "dma", dst=("acc", None), src=("xslab", None), inc=("is", 16))
    emit("v", "memset", inc=("v", 1))
    zero_v = cnt["v"]

    def stream(tg):
        if tg >= NT:
            return
        u, ti, t = tiles_flat[tg % TPL]
        p = tg % NB
        waits = []
        if tg >= NB:
            waits.append((f"g{p}", g_after_tile[tg - NB]))
        emit("x", "dma", dst=("gix", (p, t["slots"] * 8)),
             src=("gidx_slice", (t["ni_off16"], t["slots"] * 8)),
             waits=waits, inc=(f"sx{p}", 16))
        sx_after_stream[tg] = cnt[f"sx{p}"]

    for i in range(NB):
        stream(i)
    # prologue: zero bank parities for layers 0 and 1 (after streams so the
    # gather pipeline starts immediately; scatters consume these much later)
    PROLOG_Z = 128 * min(2, n_layers)
    for par in range(min(2, n_layers)):
        for gg in range(NG):
            for kk in range(C):
                emit("x", "dma", dst=("ybank_chunk", (par, gg, kk)),
                     src=("zeros", None),
                     waits=[("v", zero_v)], inc=(f"z{gg}", 16))

    def flushA(upto):
        while pendA and pendA[0][0] <= upto:
            _, fn = pendA.pop(0)
            fn()

    def flushB(upto):
        while pendB and pendB[0][0] <= upto:
            _, fn = pendB.pop(0)
            fn()

    def gflush(upto):
        while sdefer and sdefer[0][0] <= upto:
            _, fn = sdefer.pop(0)
            fn()
        while agdefer and agdefer[0][0] <= upto:
            _, fn = agdefer.pop(0)
            fn()

    ucnt = 0
    for L in range(n_layers):
        kbase = L * NG
        flushA(kbase - 1)
        gflush(ucnt)            # scatters of previous layer tail
        # AGs of previous layer (incl. group 7) must be on the gpsimd stream
        # BEFORE this layer's first gather (which waits on restripes)
        gflush(10 ** 9)
        flushB(kbase - 1)
        for gt, (u, ti, t) in enumerate(tiles_flat):
            tg = L * TPL + gt
            p = tg % NB
            uid, gg, kk = u["uid"], u["g"], u["k"]
            flushA(kbase + gg - 2)
            flushB(kbase + gg - 3)

            waits = [(f"sx{p}", sx_after_stream[tg])]
            if tg >= NB:
                waits.append(("v", v_after_tile[tg - NB]))
            if L >= 1:
                waits.append(("r0", r_after.get(("r0", L - 1), 0)))
                waits.append(("r1", r_after.get(("r1", L - 1), 0)))
            emit("g", "gather", tile=p, slots=t["slots"], layer=L,
                 bucket=kk, ni=t["slots"] * 128, queue=tg % 4, waits=waits,
                 inc=(f"g{p}", 16))
            g_after_tile[tg] = cnt[f"g{p}"]

            stream(tg + NB)
            gflush(ucnt - 1)

            waits = [(f"g{p}", g_after_tile[tg])]
            if tg == 0:
                waits.append(("is", 48))
            emit("v", "mult", tile=p, slots=t["slots"], voff=t["slot_off"],
                 waits=waits, inc=("v", 1))
            first_red = (ti == 0)
            c0 = 0
            b = t["b0"]
            for (D, nb) in t["runs"]:
                w = []
                if first_red and ucnt >= 2:
                    sp = (ucnt - 2) % 2
                    w = [(f"ss{sp}", s_after_ucnt[ucnt - 2])]
                first_red = False
                emit("v", "reduce", tile=p, c0=c0, D=D, nb=nb, b0=b,
                     yp=ucnt % 2, waits=w, inc=("v", 1))
                c0 += D * nb
                b += nb
            v_after_tile[tg] = cnt["v"]

            if ti == len(u["tiles"]) - 1:
                v_after_unit[ucnt] = cnt["v"]

                def mk_scatter(Lc, gc, kc, uc):
                    def fn():
                        sp = uc % 2
                        zval = PROLOG_Z if Lc <= 1 else \
                            PROLOG_Z + 128 * (Lc - 1)
                        waits = [("v", v_after_unit[uc]),
                                 (f"z{gc}", zval),
                                 (f"ss{sp}", cnt[f"ss{sp}"])]
                        if cnt["ss0"] == 0 and cnt["ss1"] == 0:
                            waits.append(("is", 48))
                        emit("g", "scatter", g=gc, k=kc, uid=gc * C + kc,
                             layer=Lc, yp=uc % 2, queue=sp,
                             waits=waits, inc=(f"ss{sp}", 16))
                        s_after_ucnt[uc] = cnt[f"ss{sp}"]
                    return fn

                sdefer.append((ucnt, mk_scatter(L, gg, kk, ucnt)))

                if kk == C - 1:
                    if L < n_layers - 1:
                        def mk_ag(Lc, gc):
                            def fn():
                                agk = Lc * NG + gc
                                waits = [(f"ys{agk % 2}",
                                          ys_after[(Lc, gc)])]
                                if agk >= 2:
                                    waits.append((f"r{agk % 2}",
                                                  16 * (agk // 2)))
                                emit("g", "ag", g=gc, buf=agk % 2,
                                     waits=waits, inc=(f"c{agk % 2}", 1))
                                c_after_ag[(Lc, gc)] = cnt[f"c{agk % 2}"]
                            return fn

                        agdefer.append(((L * NG + gg + 2) * C - 1,
                                        mk_ag(L, gg)))

                    def mk_A(Lc, gc):
                        def fn():
                            pa = (Lc * NG + gc) % 2
                            vbase = cnt["v"]
                            for kk2 in range(C):
                                bw = [("v", vbase if kk2 < 2
                                       else cnt["v"])]
                                if kk2 == 0:
                                    bw.append(("ss0", cnt["ss0"]))
                                    bw.append(("ss1", cnt["ss1"]))
                                    prevg = (Lc * NG + gc) - 2
                                    if prevg >= 0 and \
                                            (prevg // NG, prevg % NG) \
                                            in ys_after:
                                        bw.append(
                                            (f"ys{pa}",
                                             ys_after[(prevg // NG,
                                                       prevg % NG)]))
                                emit("x", "dma", dst=("btmp", kk2 % 2),
                                     src=("ybank_chunk", (Lc % 2, gc, kk2)),
                                     waits=bw, inc=(f"a{kk2 % 2}", 16))
                                emit("v", "banksum", g=gc, buf=kk2 % 2,
                                     ytmp=pa, first=(kk2 == 0),
                                     waits=[(f"a{kk2 % 2}",
                                             cnt[f"a{kk2 % 2}"])],
                                     inc=("v", 1))
                            emit("v", "accadd", g=gc, buf=pa, inc=("v", 1))
                            v_after_accadd[(Lc, gc)] = cnt["v"]
                            if Lc < n_layers - 1:
                                agk = Lc * NG + gc
                                emit("x", "dma", dst=("ysum_chunk", gc),
                                     src=("ytmp", pa),
                                     waits=[("v", cnt["v"])],
                                     inc=(f"ys{agk % 2}", 16))
                                ys_after[(Lc, gc)] = cnt[f"ys{agk % 2}"]
                        return fn

                    pendA.append((L * NG + gg, mk_A(L, gg)))

                    if L < n_layers - 1:
                        def mk_B(Lc, gc):
                            def fn():
                                agk = Lc * NG + gc
                                emit("x", "dma",
                                     dst=("xn_stripe", (Lc, gc)),
                                     src=("agt", agk % 2),
                                     waits=[(f"c{agk % 2}",
                                             c_after_ag[(Lc, gc)])],
                                     inc=(f"r{agk % 2}", 16))
                                r_after[(f"r{agk % 2}", Lc)] = \
                                    cnt[f"r{agk % 2}"]
                                if Lc + 2 < n_layers:
                                    for kk2 in range(C):
                                        bw = ([(f"z{gc}", cnt[f"z{gc}"])]
                                              if kk2 == 0 else [])
                                        emit("x", "dma",
                                             dst=("ybank_chunk",
                                                  (Lc % 2, gc, kk2)),
                                             src=("zeros", None),
                                             waits=bw,
                                             inc=(f"z{gc}", 16))
                            return fn

                        pendB.append((L * NG + gg, mk_B(L, gg)))
                ucnt += 1

    gflush(ucnt)
    flushA(10 ** 9)
    gflush(10 ** 9)
    flushB(10 ** 9)
    emit("v", "scale", factor=1.0 / float(n_layers + 1), inc=("v", 1))
    emit("x", "dma", dst=("out", None), src=("acc", None),
         waits=[("v", cnt["v"])], inc=("os", 16))
    emit("x", "wait", waits=[("os", cnt["os"])])
    emit("g", "wait", waits=[("ss0", cnt["ss0"]), ("ss1", cnt["ss1"])]
         + [(f"g{i}", cnt[f"g{i}"]) for i in range(NB)])
    return dict(sched=sched, sems=SEMS)


# ---------------------------------------------------------------------------
# bass graph
# ---------------------------------------------------------------------------

def build_nc(plan, cfg, n_layers, detect_races=True):
    import concourse.bass as bass
    import concourse.bacc as bacc
    import concourse.mybir as mybir
    from concourse.library_config import mlp

    C, NG, GR, RANKS, BL, SLAB, SLABP, EMB, WMAX, N = (
        cfg["C"], cfg["NG"], cfg["GR"], cfg["RANKS"], cfg["BL"], cfg["SLAB"],
        cfg["SLABP"], cfg["EMB"], cfg["WMAX"], cfg["N"])
    NU = NG * C
    FP, I16 = mybir.dt.float32, mybir.dt.int16
    TOTS, TOTNI16 = plan["tot_slots"], plan["tot_ni16"]
    PF_ACC = SLABP * EMB // 128
    PF_CH = GR * EMB // 128

    plan_s = build_sched(plan, cfg, n_layers)
    sched, sem_names = plan_s["sched"], plan_s["sems"]

    nc = bacc.Bacc("TRN2", detect_race_conditions=detect_races,
                   num_swdge_queues=4)
    x_in = nc.declare_dram_parameter("xfull", [N, EMB], FP, isOutput=False)
    xslab_in = nc.declare_dram_parameter("xslab", [SLABP, EMB], FP,
                                         isOutput=False)
    gidx_in = nc.declare_dram_parameter("gidx", [128, TOTNI16], I16,
                                        isOutput=False)
    gval_in = nc.declare_dram_parameter("gval", [128, TOTS], FP,
                                        isOutput=False)
    sidx_in = nc.declare_dram_parameter("sidx", [128, NU * (RANKS // 16)], I16,
                                        isOutput=False)
    out_ext = nc.declare_dram_parameter("out", [128, PF_ACC], FP,
                                        isOutput=True)

    xN = [nc.dram_tensor(f"xn{i}", [C * SLABP, EMB], FP) for i in range(2)]
    ybank = [nc.dram_tensor(f"ybank{i}", [C * SLABP, EMB], FP)
             for i in range(2)]
    ysum = nc.dram_tensor("ysum", [SLABP, EMB], FP)
    agt = [nc.dram_tensor(f"agt{i}", [C * GR, EMB], FP, addr_space="Shared")
           for i in range(2)]

    from contextlib import ExitStack
    NB = cfg["NB"]
    stack = ExitStack()
    with (
        stack,
        nc.Block() as block,
    ):
        tiles = [stack.enter_context(  # noqa: ANT232
            nc.sbuf_tensor(f"tile{i}", [128, WMAX, EMB], FP))
            for i in range(NB)]
        gixs = [stack.enter_context(  # noqa: ANT232
            nc.sbuf_tensor(f"gix{i}", [128, WMAX * 8], I16))
            for i in range(NB)]
        gval_s = stack.enter_context(nc.sbuf_tensor("gval_s", [128, TOTS], FP))
        sidx_s = stack.enter_context(
            nc.sbuf_tensor("sidx_s", [128, NU * (RANKS // 16)], I16))
        yperms = [stack.enter_context(  # noqa: ANT232
            nc.sbuf_tensor(f"yperm{i}", [128, BL, EMB], FP)) for i in range(2)]
        zeros = stack.enter_context(nc.sbuf_tensor("zeros", [128, PF_CH], FP))
        acc = stack.enter_context(nc.sbuf_tensor("acc", [128, PF_ACC], FP))
        ytmps = [stack.enter_context(  # noqa: ANT232
            nc.sbuf_tensor(f"ytmp{i}", [128, PF_CH], FP)) for i in range(2)]
        btmps = [stack.enter_context(  # noqa: ANT232
            nc.sbuf_tensor(f"btmp{i}", [128, PF_CH], FP)) for i in range(2)]
        SEM = {s: stack.enter_context(nc.semaphore(s)) for s in sem_names}  # noqa: ANT232

        def wv(e, op):
            for sname, val in op["waits"]:
                e.wait_ge(SEM[sname], val)

        def ap_of(spec):
            name, arg = spec
            if name == "gval_in":
                return gval_in[:, :]
            if name == "sidx_in":
                return sidx_in[:, :]
            if name == "gval_s":
                return gval_s[:, :]
            if name == "sidx_s":
                return sidx_s[:, :]
            if name == "xslab":
                # chunk-wise layout: acc[p, g*PF_CH+u] = xslab el g*GR*EMB + p*PF_CH + u
                return xslab_in[:, :].flatten().rearrange(
                    "(g p u) -> p g u", g=NG, p=128, u=PF_CH)
            if name == "acc":
                return acc[:, :]
            if name == "out":
                return out_ext[:, :]
            if name == "zeros":
                return zeros[:, :]
            if name == "ybank_chunk":
                par, gg2, kk2 = arg
                r0 = kk2 * SLABP + gg2 * GR
                return ybank[par][r0:r0 + GR, :]
            if name == "ysum_chunk":
                return ysum[arg * GR:(arg + 1) * GR, :]
            if name == "ytmp":
                return ytmps[arg][:, :]
            if name == "btmp":
                return btmps[arg][:, :]
            if name == "gix":
                buf, w16 = arg
                return gixs[buf][:, :w16]
            if name == "gidx_slice":
                off, w16 = arg
                return gidx_in[:, off:off + w16]
            if name == "agt":
                return agt[arg][:, :]
            if name == "xn_stripe":
                Lc, gc = arg
                ap = xN[(Lc + 1) % 2][:, :].rearrange("(c r) e -> c r e", c=C)
                return ap[:, gc * GR:(gc + 1) * GR, :]
            raise KeyError(name)

        @block.gpsimd
        def _(e: bass.BassGpSimd):
            for op in sched:
                if op["eng"] != "g":
                    continue
                if op["kind"] == "lib":
                    e.load_library(mlp)
                    continue
                wv(e, op)
                if op["kind"] == "gather":
                    L, kk, ni, slots = (op["layer"], op["bucket"], op["ni"],
                                        op["slots"])
                    if L == 0:
                        table = x_in[kk * SLAB:(kk + 1) * SLAB, :]
                    else:
                        table = xN[L % 2][kk * SLABP:(kk + 1) * SLABP, :]
                    e.dma_gather(
                        tiles[op["tile"]][:, :slots, :], table,
                        gixs[op["tile"]][:, :ni // 16], ni, ni, EMB,
                        single_packet=False, queue_num=op["queue"],
                    ).then_inc(SEM[op["inc_sem"]], 16)
                elif op["kind"] == "scatter":
                    uid, gg, kk = op["uid"], op["g"], op["k"]
                    sl = sidx_s[:, uid * (RANKS // 16):
                                (uid + 1) * (RANKS // 16)]
                    r0 = kk * SLABP + gg * GR
                    e.dma_scatter_add(
                        ybank[op["layer"] % 2][r0:r0 + GR, :],
                        yperms[op["yp"]][:, :, :],
                        sl, RANKS, GR, EMB, single_packet=False,
                        queue_num=op["queue"],
                    ).then_inc(SEM[op["inc_sem"]], 16)
                elif op["kind"] == "ag":
                    e.collective_compute(
                        "AllGather", mybir.AluOpType.bypass,
                        replica_groups=[list(range(C))],
                        ins=[ysum[op["g"] * GR:(op["g"] + 1) * GR, :]],
                        outs=[agt[op["buf"]][:, :]],
                    ).then_inc(SEM[op["inc_sem"]], 1)

        @block.vector
        def _(e: bass.BassVectorEngine):
            for op in sched:
                if op["eng"] != "v":
                    continue
                wv(e, op)
                if op["kind"] == "memset":
                    e.memset(zeros[:, :], 0.0).then_inc(SEM["v"], 1)
                elif op["kind"] == "mult":
                    t = tiles[op["tile"]]
                    slots, voff = op["slots"], op["voff"]
                    vb = gval_s[:, voff:voff + slots].unsqueeze(-1) \
                        .broadcast_to((128, slots, EMB))
                    e.tensor_tensor(t[:, :slots, :], t[:, :slots, :], vb,
                                    mybir.AluOpType.mult).then_inc(SEM["v"], 1)
                elif op["kind"] == "reduce":
                    t = tiles[op["tile"]]
                    c0, D, nb, b0 = op["c0"], op["D"], op["nb"], op["b0"]
                    src = t[:, c0:c0 + nb * D, :].rearrange(
                        "p (nb d) e -> p nb e d", nb=nb, d=D)
                    e.tensor_reduce(yperms[op["yp"]][:, b0:b0 + nb, :], src,
                                    mybir.AxisListType.X,
                                    mybir.AluOpType.add).then_inc(SEM["v"], 1)
                elif op["kind"] == "banksum":
                    yt = ytmps[op["ytmp"]][:, :]
                    bt = btmps[op["buf"]][:, :]
                    if op["first"]:
                        e.tensor_copy(yt, bt).then_inc(SEM["v"], 1)
                    else:
                        e.tensor_tensor(yt, yt, bt,
                                        mybir.AluOpType.add
                                        ).then_inc(SEM["v"], 1)
                elif op["kind"] == "accadd":
                    sl = acc[:, op["g"] * PF_CH:(op["g"] + 1) * PF_CH]
                    e.tensor_tensor(sl, sl, ytmps[op["buf"]][:, :],
                                    mybir.AluOpType.add).then_inc(SEM["v"], 1)
                elif op["kind"] == "scale":
                    e.tensor_scalar_mul(acc[:, :], acc[:, :],
                                        op["factor"]).then_inc(SEM["v"], 1)

        @block.sync
        def _(e):
            for op in sched:
                if op["eng"] != "x":
                    continue
                wv(e, op)
                if op["kind"] == "dma":
                    e.dma_start(out=ap_of(op["dst"]), in_=ap_of(op["src"])
                                ).then_inc(SEM[op["inc_sem"]], 16)

    nc.compile()
    return nc


# ---------------------------------------------------------------------------
# host entry
# ---------------------------------------------------------------------------

def _prep_inputs(user_emb, item_emb, adj_row, adj_col, adj_vals, cfg):
    C, SLAB, SLABP, EMB, N = (cfg["C"], cfg["SLAB"], cfg["SLABP"], cfg["EMB"],
                              cfg["N"])
    x = np.ascontiguousarray(
        np.concatenate([np.asarray(user_emb), np.asarray(item_emb)], axis=0)
        .astype(np.float32))
    plan = build_plan(adj_row, adj_col, adj_vals, cfg)
    in_maps = []
    for c in range(C):
        xs = np.zeros((SLABP, EMB), dtype=np.float32)
        xs[:SLAB] = x[c * SLAB:(c + 1) * SLAB]
        in_maps.append({
            "xfull": x,
            "xslab": xs,
            "gidx": np.ascontiguousarray(plan["gidx"][c]),
            "gval": np.ascontiguousarray(plan["gval"][c]),
            "sidx": np.ascontiguousarray(plan["sidx"][c]),
        })
    return plan, in_maps


def _unscramble(outs, cfg):
    C, NG, GR, SLAB, SLABP, EMB = (cfg["C"], cfg["NG"], cfg["GR"], cfg["SLAB"],
                                   cfg["SLABP"], cfg["EMB"])
    full = np.empty((cfg["N"], EMB), dtype=np.float32)
    for c in range(C):
        a = outs[c].reshape(128, NG, GR * EMB // 128)
        for g in range(NG):
            chunk = a[:, g, :].reshape(-1).reshape(GR, EMB)
            r0 = c * SLAB + g * GR
            nreal = min(GR, SLAB - g * GR)
            full[r0:r0 + nreal] = chunk[:nreal]
    return full


_last_exec_ns = None


def _install_ntff_hook():
    """The agent image's antenv lacks axon_hooks; synthesize it and register
    the ctypes NTFF profiling hook so trace=True yields exec_time_ns."""
    import types
    try:
        import antenv.axon_hooks  # noqa: F401
        return
    except ImportError:
        pass
    try:
        mod = types.ModuleType("antenv.axon_hooks")
        _h = [None]
        mod.get_axon_ntff_profile_hook = lambda: _h[0]
        mod.set_axon_ntff_profile_hook = lambda hk: _h.__setitem__(0, hk)
        sys.modules["antenv.axon_hooks"] = mod
        import antenv
        antenv.axon_hooks = mod
        if "/root/.axon_site" not in sys.path:
            sys.path.append("/root/.axon_site")
        from trn_agent_boot.trn_boot import _ntff_profile_via_ctypes
        hk = _ntff_profile_via_ctypes("/opt/axon/libaxon_pjrt.so")
        mod.set_axon_ntff_profile_hook(hk)
    except Exception as ex:  # degrade to no tracing
        print(f"[kernel] ntff hook install failed: {ex}", flush=True)


def kernel(user_emb, item_emb, adj_row, adj_col, adj_vals, n_layers,
           trace=True):
    global _last_exec_ns
    import time
    from concourse.bass_utils import run_bass_kernel_spmd

    t0 = time.time()
    _install_ntff_hook()
    n_layers = int(np.asarray(n_layers))
    cfg = make_cfg()
    plan, in_maps = _prep_inputs(user_emb, item_emb, adj_row, adj_col,
                                 adj_vals, cfg)
    t1 = time.time()
    nc = build_nc(plan, cfg, n_layers, detect_races=False)
    t2 = time.time()
    res = run_bass_kernel_spmd(nc, in_maps, list(range(cfg["C"])),
                               trace=trace)
    t3 = time.time()
    print(f"[kernel] prep {t1-t0:.1f}s build {t2-t1:.1f}s run {t3-t2:.1f}s",
          flush=True)
    _last_exec_ns = res.exec_time_ns
    outs = [res.results[c]["out"] for c in range(cfg["C"])]
    full = _unscramble(outs, cfg)
    return full[:USER_NUM], full[USER_NUM:]


# revision 33
# speedup vs baseline: 1.9743x; 1.0092x over previous
"""LightGCN-style GNN message passing (n_layers x SpMM + running mean) on 8 TRN2
NeuronCores.

Row-parallel SpMM (1D graph partition):
  - Core c owns dest-node slab [c*SLAB, (c+1)*SLAB).
  - Src nodes are split into C buckets == the C slabs, so every dma_gather index
    is slab-local (< 18750, fits the int16 index requirement).
  - Work unit = (dest group g of GR nodes, src bucket k).  Host builds a padded
    CSR per unit with nodes degree-sorted into blocks of 128 and a GLOBAL
    (core-max) block-degree profile so the SPMD graph is identical on all cores.
  - Per unit: per-edge dma_gather (f32 rows, 256 B) from the bucket table ->
    DVE multiply by vals -> DVE strided segment-reduce per equal-degree block
    run -> dma_scatter_add merges the bucket partial into the HBM y slab.
  - Per dest group g one small AllGather broadcasts all cores' group-g rows;
    a local re-stripe DMA lays them out slab-contiguously as next layer's
    gather tables.  The C small AllGathers per layer pipeline behind compute.
  - acc (running sum over layer outputs) lives in SBUF f32; final scale by
    1/(n_layers+1); host unscrambles the partition-major output layout.
"""

import sys
import numpy as np

if "/opt/trn_rl_repo" not in sys.path:
    sys.path.insert(0, "/opt/trn_rl_repo")

USER_NUM = 100000
ITEM_NUM = 50000


def make_cfg(n_nodes=150000, emb=64, n_cores=8, n_groups=8, wmax=32, nb=8):
    slab = n_nodes // n_cores
    assert slab * n_cores == n_nodes
    gr = -(-slab // n_groups)
    ranks = -(-gr // 128) * 128
    return dict(N=n_nodes, EMB=emb, C=n_cores, NG=n_groups, GR=gr, RANKS=ranks,
                BL=ranks // 128, SLAB=slab, SLABP=gr * n_groups, WMAX=wmax,
                NB=nb)


# ---------------------------------------------------------------------------
# host-side plan
# ---------------------------------------------------------------------------

def build_plan(rows, cols, vals, cfg):
    C, NG, GR, RANKS, BL, SLAB, WMAX = (
        cfg["C"], cfg["NG"], cfg["GR"], cfg["RANKS"], cfg["BL"], cfg["SLAB"],
        cfg["WMAX"])
    NU = NG * C

    rows = np.asarray(rows, dtype=np.int64)
    cols = np.asarray(cols, dtype=np.int64)
    vals = np.asarray(vals, dtype=np.float32)
    c = rows // SLAB
    loc = rows - c * SLAB
    g = np.minimum(loc // GR, NG - 1)
    grow = loc - g * GR
    k = cols // SLAB
    colloc = (cols - k * SLAB).astype(np.int16)

    cu = (c * NU + g * C + k).astype(np.int64)
    nkey = cu * GR + grow

    deg = np.bincount(nkey, minlength=C * NU * GR).reshape(C * NU, GR)
    order = np.argsort(-deg, axis=1, kind="stable")          # rank -> node id
    rank_of = np.empty_like(order)
    np.put_along_axis(rank_of, order,
                      np.broadcast_to(np.arange(GR), (C * NU, GR)), axis=1)

    ds = np.zeros((C * NU, RANKS), dtype=np.int64)
    ds[:, :GR] = np.take_along_axis(deg, order, axis=1)
    bmax = ds.reshape(C * NU, BL, 128).max(axis=2)
    Dprof = bmax.reshape(C, NU, BL).max(axis=0)              # [NU, BL]
    assert Dprof.max() <= WMAX, f"block degree {Dprof.max()} > WMAX {WMAX}"

    # tile packing + global slot offsets per block
    glob_blk_slot = np.zeros((NU, BL), dtype=np.int64)
    units = []
    tot_slots = 0
    for u in range(NU):
        gg, kk = divmod(u, C)
        nbl = int((Dprof[u] > 0).sum())      # zero-D blocks only at the end
        tiles = []
        b = 0
        while b < nbl:
            slots = 0
            runs = []
            tile_off = tot_slots
            b0 = b
            while b < nbl and slots + int(Dprof[u, b]) <= WMAX:
                D = int(Dprof[u, b])
                glob_blk_slot[u, b] = tile_off + slots
                if runs and runs[-1][0] == D:
                    runs[-1][1] += 1
                else:
                    runs.append([D, 1])
                slots += D
                b += 1
            tiles.append(dict(slots=slots, runs=[tuple(r) for r in runs],
                              b0=b0, slot_off=tile_off))
            tot_slots += slots
        units.append(dict(g=gg, k=kk, uid=u, tiles=tiles))

    # within-node edge counter j
    es = np.argsort(nkey, kind="stable")
    nk_s = nkey[es]
    first = np.r_[0, np.flatnonzero(np.diff(nk_s)) + 1]
    starts = np.zeros(len(nk_s), dtype=np.int64)
    starts[first] = first
    starts = np.maximum.accumulate(starts)
    j = np.empty_like(starts)
    j[es] = np.arange(len(nk_s)) - starts

    r_e = rank_of[cu, grow]
    u_e = cu % NU
    slot_e = glob_blk_slot[u_e, r_e // 128] + j
    pos = slot_e * 128 + (r_e % 128)

    gidx_lin = np.zeros((C, tot_slots * 128), dtype=np.int16)
    gval_lin = np.zeros((C, tot_slots * 128), dtype=np.float32)
    gidx_lin[c, pos] = colloc
    gval_lin[c, pos] = vals

    gval_w = np.ascontiguousarray(
        gval_lin.reshape(C, tot_slots, 128).transpose(0, 2, 1))

    # wrap gather idx per tile -> [C, 128, tot_slots*8]
    gidx_w = np.zeros((C, 128, tot_slots * 8), dtype=np.int16)
    off16 = 0
    for u in range(NU):
        for t in units[u]["tiles"]:
            ni = t["slots"] * 128
            a = t["slot_off"] * 128
            seg = gidx_lin[:, a:a + ni].reshape(C, ni // 16, 16)
            gidx_w[:, :16, off16:off16 + ni // 16] = seg.transpose(0, 2, 1)
            t["ni_off16"] = off16
            off16 += ni // 16
    gidx_w[:, 16:, :] = np.tile(gidx_w[:, :16, :], (1, 7, 1))

    # scatter idx per unit
    sidx_lin = np.full((C, NU, RANKS), -1, dtype=np.int16)
    sidx_lin[:, :, :GR] = order.reshape(C, NU, GR).astype(np.int16)
    seg = sidx_lin.reshape(C, NU, RANKS // 16, 16).transpose(0, 1, 3, 2)
    sidx_w = np.broadcast_to(seg[:, :, None, :, :],
                             (C, NU, 8, 16, RANKS // 16))
    sidx_w = np.ascontiguousarray(
        sidx_w.reshape(C, NU, 128, RANKS // 16).transpose(0, 2, 1, 3)
        .reshape(C, 128, NU * (RANKS // 16)))

    return dict(units=units, tot_slots=tot_slots, tot_ni16=off16,
                gidx=gidx_w, gval=gval_w, sidx=sidx_w)


# ---------------------------------------------------------------------------
# schedule (engine-op list with cumulative semaphore targets)
# ---------------------------------------------------------------------------

def build_sched(plan, cfg, n_layers):
    """Engine-op schedule.  Invariants: every semaphore wait value is a
    total-so-far on that sem at emit time, and any two DMAs sharing a sem are
    ordered by a consumer dependency (or waited as a full-burst total), so
    thresholds are race-free.  Gathers rotate over NB buffers/sems and SWDGE
    queues 0-3; scatters write disjoint per-bucket HBM bank regions
    (race-free) on queues 0/1 with 2 rotating sems.  Bank partials are summed
    on DVE per dest group (boundaryA, lag 2 groups), the AllGather is issued
    on gpsimd during group g+2, and restripe+bank-rezero run at lag 3
    (boundaryB), so the gpsimd engine never stalls mid-pipeline."""
    C, NG, NB = cfg["C"], cfg["NG"], cfg["NB"]
    units = plan["units"]
    tiles_flat = []
    for u in units:
        for ti, t in enumerate(u["tiles"]):
            tiles_flat.append((u, ti, t))
    TPL = len(tiles_flat)
    NT = TPL * n_layers

    SEMS = (["ss0", "ss1", "c0", "c1", "is", "a0", "a1", "r0", "r1", "v",
             "os", "ys0", "ys1"]
            + [f"z{g}" for g in range(NG)]
            + [f"g{i}" for i in range(NB)] + [f"sx{i}" for i in range(NB)])
    cnt = {s: 0 for s in SEMS}
    sched = []

    def emit(eng, kind, waits=(), inc=None, **kw):
        w = {}
        for sname, val in waits:
            w[sname] = max(w.get(sname, 0), val)
        sched.append(dict(eng=eng, kind=kind,
                          waits=[(s_, v_) for s_, v_ in w.items() if v_ > 0],
                          inc_sem=inc[0] if inc else None, **kw))
        if inc:
            cnt[inc[0]] += inc[1]

    v_after_tile = {}
    g_after_tile = {}
    sx_after_stream = {}
    s_after_ucnt = {}
    v_after_unit = {}
    r_after = {}
    c_after_ag = {}
    v_after_accadd = {}
    ys_after = {}
    pendA = []      # ysum/accadd thunks, flushed at lag 2 (sync+vector)
    pendB = []      # restripe + bank-rezero thunks, lag 3 (sync)
    sdefer = []     # deferred gpsimd scatters keyed by ucnt
    agdefer = []    # deferred gpsimd AGs keyed by ucnt

    emit("g", "lib")
    emit("x", "dma", dst=("gval_s", None), src=("gval_in", None), inc=("is", 16))
    emit("x", "dma", dst=("sidx_s", None), src=("sidx_in", None), inc=("is", 16))
    emit("x", "dma", dst=("acc", None), src=("xslab", None), inc=("is", 16))
    emit("v", "memset", inc=("v", 1))
    zero_v = cnt["v"]

    def stream(tg):
        if tg >= NT:
            return
        u, ti, t = tiles_flat[tg % TPL]
        p = tg % NB
        waits = []
        if tg >= NB:
            waits.append((f"g{p}", g_after_tile[tg - NB]))
        emit("x", "dma", dst=("gix", (p, t["slots"] * 8)),
             src=("gidx_slice", (t["ni_off16"], t["slots"] * 8)),
             waits=waits, inc=(f"sx{p}", 16))
        sx_after_stream[tg] = cnt[f"sx{p}"]

    for i in range(NB):
        stream(i)
    # prologue: zero bank parities for layers 0 and 1 (after streams so the
    # gather pipeline starts immediately; scatters consume these much later)
    PROLOG_Z = 128 * min(2, n_layers)
    for par in range(min(2, n_layers)):
        for gg in range(NG):
            for kk in range(C):
                emit("x", "dma", dst=("ybank_chunk", (par, gg, kk)),
                     src=("zeros", None),
                     waits=[("v", zero_v)], inc=(f"z{gg}", 16))

    def flushA(upto):
        while pendA and pendA[0][0] <= upto:
            _, fn = pendA.pop(0)
            fn()

    def flushB(upto):
        while pendB and pendB[0][0] <= upto:
            _, fn = pendB.pop(0)
            fn()

    def gflush(upto):
        while sdefer and sdefer[0][0] <= upto:
            _, fn = sdefer.pop(0)
            fn()
        while agdefer and agdefer[0][0] <= upto:
            _, fn = agdefer.pop(0)
            fn()

    ucnt = 0
    for L in range(n_layers):
        kbase = L * NG
        flushA(kbase - 1)
        gflush(ucnt)            # scatters of previous layer tail
        # AGs of previous layer (incl. group 7) must be on the gpsimd stream
        # BEFORE this layer's first gather (which waits on restripes)
        gflush(10 ** 9)
        flushB(kbase - 1)
        for gt, (u, ti, t) in enumerate(tiles_flat):
            tg = L * TPL + gt
            p = tg % NB
            uid, gg, kk = u["uid"], u["g"], u["k"]
            flushA(kbase + gg - 2)
            flushB(kbase + gg - 3)

            waits = [(f"sx{p}", sx_after_stream[tg])]
            if tg >= NB:
                waits.append(("v", v_after_tile[tg - NB]))
            if L >= 1:
                waits.append(("r0", r_after.get(("r0", L - 1), 0)))
                waits.append(("r1", r_after.get(("r1", L - 1), 0)))
            emit("g", "gather", tile=p, slots=t["slots"], layer=L,
                 bucket=kk, ni=t["slots"] * 128, queue=tg % 4, waits=waits,
                 inc=(f"g{p}", 16))
            g_after_tile[tg] = cnt[f"g{p}"]

            stream(tg + NB)
            gflush(ucnt - 1)

            waits = [(f"g{p}", g_after_tile[tg])]
            if tg == 0:
                waits.append(("is", 48))
            emit("v", "mult", tile=p, slots=t["slots"], voff=t["slot_off"],
                 waits=waits, inc=("v", 1))
            first_red = (ti == 0)
            c0 = 0
            b = t["b0"]
            for (D, nb) in t["runs"]:
                w = []
                if first_red and ucnt >= 2:
                    sp = (ucnt - 2) % 2
                    w = [(f"ss{sp}", s_after_ucnt[ucnt - 2])]
                first_red = False
                emit("v", "reduce", tile=p, c0=c0, D=D, nb=nb, b0=b,
                     yp=ucnt % 2, waits=w, inc=("v", 1))
                c0 += D * nb
                b += nb
            v_after_tile[tg] = cnt["v"]

            if ti == len(u["tiles"]) - 1:
                v_after_unit[ucnt] = cnt["v"]

                def mk_scatter(Lc, gc, kc, uc):
                    def fn():
                        sp = uc % 2
                        zval = PROLOG_Z if Lc <= 1 else \
                            PROLOG_Z + 128 * (Lc - 1)
                        waits = [("v", v_after_unit[uc]),
                                 (f"z{gc}", zval),
                                 (f"ss{sp}", cnt[f"ss{sp}"])]
                        if cnt["ss0"] == 0 and cnt["ss1"] == 0:
                            waits.append(("is", 48))
                        emit("g", "scatter", g=gc, k=kc, uid=gc * C + kc,
                             layer=Lc, yp=uc % 2, queue=sp,
                             waits=waits, inc=(f"ss{sp}", 16))
                        s_after_ucnt[uc] = cnt[f"ss{sp}"]
                    return fn

                sdefer.append((ucnt, mk_scatter(L, gg, kk, ucnt)))

                if kk == C - 1:
                    if L < n_layers - 1:
                        def mk_ag(Lc, gc):
                            def fn():
                                agk = Lc * NG + gc
                                waits = [(f"ys{agk % 2}",
                                          ys_after[(Lc, gc)])]
                                if agk >= 2:
                                    waits.append((f"r{agk % 2}",
                                                  16 * (agk // 2)))
                                emit("g", "ag", g=gc, buf=agk % 2,
                                     waits=waits, inc=(f"c{agk % 2}", 1))
                                c_after_ag[(Lc, gc)] = cnt[f"c{agk % 2}"]
                            return fn

                        agdefer.append(((L * NG + gg + 2) * C - 1,
                                        mk_ag(L, gg)))

                    def mk_A(Lc, gc):
                        def fn():
                            pa = (Lc * NG + gc) % 2
                            vbase = cnt["v"]
                            for kk2 in range(C):
                                bw = [("v", vbase if kk2 < 2
                                       else cnt["v"])]
                                if kk2 == 0:
                                    bw.append(("ss0", cnt["ss0"]))
                                    bw.append(("ss1", cnt["ss1"]))
                                    prevg = (Lc * NG + gc) - 2
                                    if prevg >= 0 and \
                                            (prevg // NG, prevg % NG) \
                                            in ys_after:
                                        bw.append(
                                            (f"ys{pa}",
                                             ys_after[(prevg // NG,
                                                       prevg % NG)]))
                                emit("x", "dma", dst=("btmp", kk2 % 2),
                                     src=("ybank_chunk", (Lc % 2, gc, kk2)),
                                     waits=bw, inc=(f"a{kk2 % 2}", 16))
                                emit("v", "banksum", g=gc, buf=kk2 % 2,
                                     ytmp=pa, first=(kk2 == 0),
                                     waits=[(f"a{kk2 % 2}",
                                             cnt[f"a{kk2 % 2}"])],
                                     inc=("v", 1))
                            emit("v", "accadd", g=gc, buf=pa, inc=("v", 1))
                            v_after_accadd[(Lc, gc)] = cnt["v"]
                            if Lc < n_layers - 1:
                                agk = Lc * NG + gc
                                emit("x", "dma", dst=("ysum_chunk", gc),
                                     src=("ytmp", pa),
                                     waits=[("v", cnt["v"])],
                                     inc=(f"ys{agk % 2}", 16))
                                ys_after[(Lc, gc)] = cnt[f"ys{agk % 2}"]
                        return fn

                    pendA.append((L * NG + gg, mk_A(L, gg)))

                    if L < n_layers - 1:
                        def mk_B(Lc, gc):
                            def fn():
                                agk = Lc * NG + gc
                                emit("x", "dma",
                                     dst=("xn_stripe", (Lc, gc)),
                                     src=("agt", agk % 2),
                                     waits=[(f"c{agk % 2}",
                                             c_after_ag[(Lc, gc)])],
                                     inc=(f"r{agk % 2}", 16))
                                r_after[(f"r{agk % 2}", Lc)] = \
                                    cnt[f"r{agk % 2}"]
                                if Lc + 2 < n_layers:
                                    for kk2 in range(C):
                                        bw = ([(f"z{gc}", cnt[f"z{gc}"])]
                                              if kk2 == 0 else [])
                                        emit("x", "dma",
                                             dst=("ybank_chunk",
                                                  (Lc % 2, gc, kk2)),
                                             src=("zeros", None),
                                             waits=bw,
                                             inc=(f"z{gc}", 16))
                            return fn

                        pendB.append((L * NG + gg, mk_B(L, gg)))
                ucnt += 1

    gflush(ucnt)
    flushA(10 ** 9)
    gflush(10 ** 9)
    flushB(10 ** 9)
    emit("v", "scale", factor=1.0 / float(n_layers + 1), inc=("v", 1))
    emit("x", "dma", dst=("out", None), src=("acc", None),
         waits=[("v", cnt["v"])], inc=("os", 16))
    emit("x", "wait", waits=[("os", cnt["os"])])
    emit("g", "wait", waits=[("ss0", cnt["ss0"]), ("ss1", cnt["ss1"])]
         + [(f"g{i}", cnt[f"g{i}"]) for i in range(NB)])
    return dict(sched=sched, sems=SEMS)


# ---------------------------------------------------------------------------
# bass graph
# ---------------------------------------------------------------------------

def build_nc(plan, cfg, n_layers, detect_races=True):
    import concourse.bass as bass
    import concourse.bacc as bacc
    import concourse.mybir as mybir
    from concourse.library_config import mlp

    C, NG, GR, RANKS, BL, SLAB, SLABP, EMB, WMAX, N = (
        cfg["C"], cfg["NG"], cfg["GR"], cfg["RANKS"], cfg["BL"], cfg["SLAB"],
        cfg["SLABP"], cfg["EMB"], cfg["WMAX"], cfg["N"])
    NU = NG * C
    FP, I16 = mybir.dt.float32, mybir.dt.int16
    TOTS, TOTNI16 = plan["tot_slots"], plan["tot_ni16"]
    PF_ACC = SLABP * EMB // 128
    PF_CH = GR * EMB // 128

    plan_s = build_sched(plan, cfg, n_layers)
    sched, sem_names = plan_s["sched"], plan_s["sems"]

    nc = bacc.Bacc("TRN2", detect_race_conditions=detect_races,
                   num_swdge_queues=4)
    x_in = nc.declare_dram_parameter("xfull", [N, EMB], FP, isOutput=False)
    xslab_in = nc.declare_dram_parameter("xslab", [SLABP, EMB], FP,
                                         isOutput=False)
    gidx_in = nc.declare_dram_parameter("gidx", [128, TOTNI16], I16,
                                        isOutput=False)
    gval_in = nc.declare_dram_parameter("gval", [128, TOTS], FP,
                                        isOutput=False)
    sidx_in = nc.declare_dram_parameter("sidx", [128, NU * (RANKS // 16)], I16,
                                        isOutput=False)
    out_ext = nc.declare_dram_parameter("out", [128, PF_ACC], FP,
                                        isOutput=True)

    xN = [nc.dram_tensor(f"xn{i}", [C * SLABP, EMB], FP) for i in range(2)]
    ybank = [nc.dram_tensor(f"ybank{i}", [C * SLABP, EMB], FP)
             for i in range(2)]
    ysum = nc.dram_tensor("ysum", [SLABP, EMB], FP)
    agt = [nc.dram_tensor(f"agt{i}", [C * GR, EMB], FP, addr_space="Shared")
           for i in range(2)]

    from contextlib import ExitStack
    NB = cfg["NB"]
    stack = ExitStack()
    with (
        stack,
        nc.Block() as block,
    ):
        tiles = [stack.enter_context(  # noqa: ANT232
            nc.sbuf_tensor(f"tile{i}", [128, WMAX, EMB], FP))
            for i in range(NB)]
        gixs = [stack.enter_context(  # noqa: ANT232
            nc.sbuf_tensor(f"gix{i}", [128, WMAX * 8], I16))
            for i in range(NB)]
        gval_s = stack.enter_context(nc.sbuf_tensor("gval_s", [128, TOTS], FP))
        sidx_s = stack.enter_context(
            nc.sbuf_tensor("sidx_s", [128, NU * (RANKS // 16)], I16))
        yperms = [stack.enter_context(  # noqa: ANT232
            nc.sbuf_tensor(f"yperm{i}", [128, BL, EMB], FP)) for i in range(2)]
        zeros = stack.enter_context(nc.sbuf_tensor("zeros", [128, PF_CH], FP))
        acc = stack.enter_context(nc.sbuf_tensor("acc", [128, PF_ACC], FP))
        ytmps = [stack.enter_context(  # noqa: ANT232
            nc.sbuf_tensor(f"ytmp{i}", [128, PF_CH], FP)) for i in range(2)]
        btmps = [stack.enter_context(  # noqa: ANT232
            nc.sbuf_tensor(f"btmp{i}", [128, PF_CH], FP)) for i in range(2)]
        SEM = {s: stack.enter_context(nc.semaphore(s)) for s in sem_names}  # noqa: ANT232

        def wv(e, op):
            for sname, val in op["waits"]:
                e.wait_ge(SEM[sname], val)

        def ap_of(spec):
            name, arg = spec
            if name == "gval_in":
                return gval_in[:, :]
            if name == "sidx_in":
                return sidx_in[:, :]
            if name == "gval_s":
                return gval_s[:, :]
            if name == "sidx_s":
                return sidx_s[:, :]
            if name == "xslab":
                # chunk-wise layout: acc[p, g*PF_CH+u] = xslab el g*GR*EMB + p*PF_CH + u
                return xslab_in[:, :].flatten().rearrange(
                    "(g p u) -> p g u", g=NG, p=128, u=PF_CH)
            if name == "acc":
                return acc[:, :]
            if name == "out":
                return out_ext[:, :]
            if name == "zeros":
                return zeros[:, :]
            if name == "ybank_chunk":
                par, gg2, kk2 = arg
                r0 = kk2 * SLABP + gg2 * GR
                return ybank[par][r0:r0 + GR, :]
            if name == "ysum_chunk":
                return ysum[arg * GR:(arg + 1) * GR, :]
            if name == "ytmp":
                return ytmps[arg][:, :]
            if name == "btmp":
                return btmps[arg][:, :]
            if name == "gix":
                buf, w16 = arg
                return gixs[buf][:, :w16]
            if name == "gidx_slice":
                off, w16 = arg
                return gidx_in[:, off:off + w16]
            if name == "agt":
                return agt[arg][:, :]
            if name == "xn_stripe":
                Lc, gc = arg
                ap = xN[(Lc + 1) % 2][:, :].rearrange("(c r) e -> c r e", c=C)
                return ap[:, gc * GR:(gc + 1) * GR, :]
            raise KeyError(name)

        @block.gpsimd
        def _(e: bass.BassGpSimd):
            for op in sched:
                if op["eng"] != "g":
                    continue
                if op["kind"] == "lib":
                    e.load_library(mlp)
                    continue
                wv(e, op)
                if op["kind"] == "gather":
                    L, kk, ni, slots = (op["layer"], op["bucket"], op["ni"],
                                        op["slots"])
                    if L == 0:
                        table = x_in[kk * SLAB:(kk + 1) * SLAB, :]
                    else:
                        table = xN[L % 2][kk * SLABP:(kk + 1) * SLABP, :]
                    e.dma_gather(
                        tiles[op["tile"]][:, :slots, :], table,
                        gixs[op["tile"]][:, :ni // 16], ni, ni, EMB,
                        single_packet=False, queue_num=op["queue"],
                    ).then_inc(SEM[op["inc_sem"]], 16)
                elif op["kind"] == "scatter":
                    uid, gg, kk = op["uid"], op["g"], op["k"]
                    sl = sidx_s[:, uid * (RANKS // 16):
                                (uid + 1) * (RANKS // 16)]
                    r0 = kk * SLABP + gg * GR
                    e.dma_scatter_add(
                        ybank[op["layer"] % 2][r0:r0 + GR, :],
                        yperms[op["yp"]][:, :, :],
                        sl, RANKS, GR, EMB, single_packet=False,
                        queue_num=op["queue"],
                    ).then_inc(SEM[op["inc_sem"]], 16)
                elif op["kind"] == "ag":
                    e.collective_compute(
                        "AllGather", mybir.AluOpType.bypass,
                        replica_groups=[list(range(C))],
                        ins=[ysum[op["g"] * GR:(op["g"] + 1) * GR, :]],
                        outs=[agt[op["buf"]][:, :]],
                    ).then_inc(SEM[op["inc_sem"]], 1)

        @block.vector
        def _(e: bass.BassVectorEngine):
            for op in sched:
                if op["eng"] != "v":
                    continue
                wv(e, op)
                if op["kind"] == "memset":
                    e.memset(zeros[:, :], 0.0).then_inc(SEM["v"], 1)
                elif op["kind"] == "mult":
                    t = tiles[op["tile"]]
                    slots, voff = op["slots"], op["voff"]
                    vb = gval_s[:, voff:voff + slots].unsqueeze(-1) \
                        .broadcast_to((128, slots, EMB))
                    e.tensor_tensor(t[:, :slots, :], t[:, :slots, :], vb,
                                    mybir.AluOpType.mult).then_inc(SEM["v"], 1)
                elif op["kind"] == "reduce":
                    t = tiles[op["tile"]]
                    c0, D, nb, b0 = op["c0"], op["D"], op["nb"], op["b0"]
                    src = t[:, c0:c0 + nb * D, :].rearrange(
                        "p (nb d) e -> p nb e d", nb=nb, d=D)
                    e.tensor_reduce(yperms[op["yp"]][:, b0:b0 + nb, :], src,
                                    mybir.AxisListType.X,
                                    mybir.AluOpType.add).then_inc(SEM["v"], 1)
                elif op["kind"] == "banksum":
                    yt = ytmps[op["ytmp"]][:, :]
                    bt = btmps[op["buf"]][:, :]
                    if op["first"]:
                        e.tensor_copy(yt, bt).then_inc(SEM["v"], 1)
                    else:
                        e.tensor_tensor(yt, yt, bt,
                                        mybir.AluOpType.add
                                        ).then_inc(SEM["v"], 1)
                elif op["kind"] == "accadd":
                    sl = acc[:, op["g"] * PF_CH:(op["g"] + 1) * PF_CH]
                    e.tensor_tensor(sl, sl, ytmps[op["buf"]][:, :],
                                    mybir.AluOpType.add).then_inc(SEM["v"], 1)
                elif op["kind"] == "scale":
                    e.tensor_scalar_mul(acc[:, :], acc[:, :],
                                        op["factor"]).then_inc(SEM["v"], 1)

        @block.sync
        def _(e):
            for op in sched:
                if op["eng"] != "x":
                    continue
                wv(e, op)
                if op["kind"] == "dma":
                    e.dma_start(out=ap_of(op["dst"]), in_=ap_of(op["src"])
                                ).then_inc(SEM[op["inc_sem"]], 16)

    nc.compile()
    return nc


# ---------------------------------------------------------------------------
# host entry
# ---------------------------------------------------------------------------

def _prep_inputs(user_emb, item_emb, adj_row, adj_col, adj_vals, cfg):
    C, SLAB, SLABP, EMB, N = (cfg["C"], cfg["SLAB"], cfg["SLABP"], cfg["EMB"],
                              cfg["N"])
    x = np.ascontiguousarray(
        np.concatenate([np.asarray(user_emb), np.asarray(item_emb)], axis=0)
        .astype(np.float32))
    plan = build_plan(adj_row, adj_col, adj_vals, cfg)
    in_maps = []
    for c in range(C):
        xs = np.zeros((SLABP, EMB), dtype=np.float32)
        xs[:SLAB] = x[c * SLAB:(c + 1) * SLAB]
        in_maps.append({
            "xfull": x,
            "xslab": xs,
            "gidx": np.ascontiguousarray(plan["gidx"][c]),
            "gval": np.ascontiguousarray(plan["gval"][c]),
            "sidx": np.ascontiguousarray(plan["sidx"][c]),
        })
    return plan, in_maps


def _unscramble(outs, cfg):
    C, NG, GR, SLAB, SLABP, EMB = (cfg["C"], cfg["NG"], cfg["GR"], cfg["SLAB"],
                                   cfg["SLABP"], cfg["EMB"])
    full = np.empty((cfg["N"], EMB), dtype=np.float32)
    for c in range(C):
        a = outs[c].reshape(128, NG, GR * EMB // 128)
        for g in range(NG):
            chunk = a[:, g, :].reshape(-1).reshape(GR, EMB)
            r0 = c * SLAB + g * GR
            nreal = min(GR, SLAB - g * GR)
            full[r0:r0 + nreal] = chunk[:nreal]
    return full


_last_exec_ns = None


def _install_ntff_hook():
    """The agent image's antenv lacks axon_hooks; synthesize it and register
    the ctypes NTFF profiling hook so trace=True yields exec_time_ns."""
    import types
    try:
        import antenv.axon_hooks  # noqa: F401
        return
    except ImportError:
        pass
    try:
        mod = types.ModuleType("antenv.axon_hooks")
        _h = [None]
        mod.get_axon_ntff_profile_hook = lambda: _h[0]
        mod.set_axon_ntff_profile_hook = lambda hk: _h.__setitem__(0, hk)
        sys.modules["antenv.axon_hooks"] = mod
        import antenv
        antenv.axon_hooks = mod
        if "/root/.axon_site" not in sys.path:
            sys.path.append("/root/.axon_site")
        from trn_agent_boot.trn_boot import _ntff_profile_via_ctypes
        hk = _ntff_profile_via_ctypes("/opt/axon/libaxon_pjrt.so")
        mod.set_axon_ntff_profile_hook(hk)
    except Exception as ex:  # degrade to no tracing
        print(f"[kernel] ntff hook install failed: {ex}", flush=True)


def kernel(user_emb, item_emb, adj_row, adj_col, adj_vals, n_layers,
           trace=True):
    global _last_exec_ns
    import time
    from concourse.bass_utils import run_bass_kernel_spmd

    t0 = time.time()
    _install_ntff_hook()
    n_layers = int(np.asarray(n_layers))
    cfg = make_cfg()
    plan, in_maps = _prep_inputs(user_emb, item_emb, adj_row, adj_col,
                                 adj_vals, cfg)
    t1 = time.time()
    nc = build_nc(plan, cfg, n_layers, detect_races=False)
    t2 = time.time()
    res = run_bass_kernel_spmd(nc, in_maps, list(range(cfg["C"])),
                               trace=trace)
    t3 = time.time()
    print(f"[kernel] prep {t1-t0:.1f}s build {t2-t1:.1f}s run {t3-t2:.1f}s",
          flush=True)
    _last_exec_ns = res.exec_time_ns
    import os as _os
    if _os.environ.get("DUMP_INSTS") and res.instructions_and_trace:
        import pickle
        insts = res.instructions_and_trace[0]
        try:
            with open("/tmp/insts.pkl", "wb") as f:
                pickle.dump(insts, f)
            print(f"[kernel] dumped {len(insts)} insts to /tmp/insts.pkl",
                  flush=True)
        except Exception as ex:
            print(f"[kernel] inst dump failed: {ex}", flush=True)
    outs = [res.results[c]["out"] for c in range(cfg["C"])]
    full = _unscramble(outs, cfg)
    return full[:USER_NUM], full[USER_NUM:]
